# revision 7
# baseline (speedup 1.0000x reference)
"""2-layer GCN (improved=True) + linear head + softmax on 8 Trainium2 cores.

Strategy (dest-node partitioning):
- Nodes assigned to 8 cores x 49 tiles x 128 slots via balanced bin-packing on
  in-degree (max 2176 in-edges per tile; self-loops are NOT gathered).
- Per layer: each core computes XW for its slots (node-major [n,128]),
  AllGather replicates the table to every core's HBM (Shared scratch for the
  fast collective path), then per dest-tile-pair the core gathers source rows
  with dma_gather (lo window 2048 idx / hi window 2304 idx, trailing -1
  padding is trimmed by the ucode) and scatter-adds them with one-hot matmuls
  agg^T[d, n] += G_chunk^T[d, e] @ S_chunk[e, n].
- S chunks are built ON-CHIP by the vector engine from compact per-chunk
  (dest-pos, norm) columns: S[e, n] = (iota[n] == dpos[e]) * val[e]; no dense
  S matrices ever touch HBM.
- The self-loop term (norm 2*dis^2) is one extra matmul per tile with
  lhsT = the tile's own XW output (kept in SBUF) and a diagonal S built the
  same way (dpos = arange, val = 2*dis^2).
- Gather calls rotate across all 4 SWDGE queues so descriptor generation for
  different calls can overlap on different GPSIMD core pairs.
- Head: logits = H2 @ Wlin + blin, softmax over 8 classes, on-chip.

kernel() is self-contained: host-side numpy does all graph preprocessing;
the device program is identical on all 8 cores, only data differs.
"""
import sys

sys.path.insert(0, "/opt/trn_rl_repo")

import numpy as np
import ml_dtypes

import concourse.bass as bass
import concourse.bacc as bacc
import concourse.mybir as mybir
import concourse.tile as tile
from concourse.tile_rust import add_dep_helper
from concourse.bass_utils import run_bass_kernel_spmd
from concourse.library_config import mlp

# problem constants
N = 50000
E = 800000
FIN = 512
D = 128
NCLS = 8
NCORES = 8

# sharding constants
P = 128
TILES = 49
NLOC = TILES * P            # 6272 slots per core
VTOT = NCORES * NLOC        # 50176 table rows
LO_CAP = 1152               # per-tile lo-window edge cap (9 chunks)
HI_CAP = 1152               # per-tile hi-window edge cap (9 chunks)
ECAP = LO_CAP + HI_CAP      # 2304 in-edges per tile
CLO = LO_CAP // P           # 9 lo chunks per tile
CHI = HI_CAP // P           # 9 hi chunks per tile
CPT = CLO + CHI             # 18 edge chunks per tile
TILES_A = 25                # tiles in AG phase A
ROWS_A = TILES_A * P        # 3328 rows/core in phase A
LO_LIM = NCORES * ROWS_A    # 26624: lo gathers read only the AG-A region
TILES_B = TILES - TILES_A
ROWS_B = TILES_B * P
PAIRS = (TILES + 1) // 2
DEPTH = 8                   # lo-gather software-pipeline depth (pairs)
HDEPTH = 2                  # hi-gather prefetch depth (pairs)
LO_COLS = 2 * LO_CAP // 16  # 128 int16 idx cols per pair (lo)
HI_COLS = 2 * HI_CAP // 16  # 144 int16 idx cols per pair (hi)
SCOLS = PAIRS * (2 * CPT) + TILES  # dpos/val columns (incl self diag)

TRACE = False
LAST_EXEC_NS = None
LAST_RESULT = None

_PROGRAM = None


def _build_program():
    nc = bacc.Bacc(None, target_bir_lowering=False, num_swdge_queues=4)
    f32 = mybir.dt.float32
    bf16 = mybir.dt.bfloat16

    xt_d = nc.dram_tensor("xt", [FIN, NLOC], bf16, kind="ExternalInput")
    w1_d = nc.dram_tensor("w1", [FIN, D], bf16, kind="ExternalInput")
    w2_d = nc.dram_tensor("w2", [D, D], bf16, kind="ExternalInput")
    wl_d = nc.dram_tensor("wl", [D, NCLS], bf16, kind="ExternalInput")
    b1_d = nc.dram_tensor("b1", [P, 1], f32, kind="ExternalInput")
    b2_d = nc.dram_tensor("b2", [P, 1], f32, kind="ExternalInput")
    bl_d = nc.dram_tensor("bl", [P, NCLS], f32, kind="ExternalInput")
    gidx_d = nc.dram_tensor("gidx", [P, PAIRS * (LO_COLS + HI_COLS)],
                            mybir.dt.int16, kind="ExternalInput")
    iota_d = nc.dram_tensor("iota", [P, P], bf16, kind="ExternalInput")
    dpos_d = nc.dram_tensor("dpos", [P, SCOLS], f32, kind="ExternalInput")
    sval_d = nc.dram_tensor("sval", [P, SCOLS], f32, kind="ExternalInput")
    out_d = nc.dram_tensor("probs", [NLOC, NCLS], f32, kind="ExternalOutput")

    with tile.TileContext(nc) as tc:
        lib = nc.gpsimd.load_library(mlp)
        first_gather = [True]
        qctr = [0]

        with (
            tc.tile_pool(name="const", bufs=1) as cp,
            tc.tile_pool(name="xtp", bufs=1) as xtp,
            tc.tile_pool(name="gpool", bufs=4) as gp,
            tc.tile_pool(name="spool", bufs=8) as sp,
            tc.tile_pool(name="hpool", bufs=3) as hp,
            tc.tile_pool(name="headp", bufs=3) as hdp,
            tc.tile_pool(name="xwps", bufs=2, space="PSUM") as xwps,
            tc.tile_pool(name="aggps", bufs=2, space="PSUM") as aggps,
            tc.tile_pool(name="lgps", bufs=2, space="PSUM") as lgps,
            tc.tile_pool(name="dram1", bufs=1, space="DRAM") as dr1,
            tc.tile_pool(name="dram2", bufs=1, space="DRAM") as dr2,
            tc.tile_pool(name="dram3", bufs=1, space="DRAM") as dr3,
            tc.tile_pool(name="dram4", bufs=1, space="DRAM") as dr4,
        ):
            # ---- constants to SBUF ----
            w1_sb = cp.tile([P, 4 * D], bf16)
            for k in range(4):
                nc.sync.dma_start(w1_sb[:, k * D:(k + 1) * D],
                                  w1_d[k * P:(k + 1) * P, :])
            w2_sb = cp.tile([P, D], bf16)
            nc.sync.dma_start(w2_sb[:], w2_d[:])
            wl_sb = cp.tile([P, NCLS], bf16)
            nc.sync.dma_start(wl_sb[:], wl_d[:])
            b1_sb = cp.tile([P, 1], f32)
            nc.sync.dma_start(b1_sb[:], b1_d[:])
            b2_sb = cp.tile([P, 1], f32)
            nc.sync.dma_start(b2_sb[:], b2_d[:])
            bl_sb = cp.tile([P, NCLS], f32)
            nc.sync.dma_start(bl_sb[:], bl_d[:])
            gidx_sb = cp.tile([P, PAIRS * (LO_COLS + HI_COLS)], mybir.dt.int16)
            nc.sync.dma_start(gidx_sb[:], gidx_d[:])
            iota_sb = cp.tile([P, P], bf16)
            nc.sync.dma_start(iota_sb[:], iota_d[:])
            dpos_sb = cp.tile([P, SCOLS], f32)
            nc.sync.dma_start(dpos_sb[:], dpos_d[:])
            sval_sb = cp.tile([P, SCOLS], f32)
            nc.sync.dma_start(sval_sb[:], sval_d[:])
            # per-layer local XW tables kept in SBUF for the self-loop matmul
            t_all = [cp.tile([P, NLOC], bf16, name="t_all0"),
                     cp.tile([P, NLOC], bf16, name="t_all1")]

            t_loc = [dr1.tile([NLOC, D], bf16, name="t_loc0"),
                     dr2.tile([NLOC, D], bf16, name="t_loc1")]
            t_fullA = [dr3.tile([LO_LIM, D], bf16, name="t_fullA0", tag="a0"),
                       dr3.tile([LO_LIM, D], bf16, name="t_fullA1", tag="a1")]
            t_fullB = [dr4.tile([VTOT - LO_LIM, D], bf16, name="t_fullB0",
                                tag="b0"),
                       dr4.tile([VTOT - LO_LIM, D], bf16, name="t_fullB1",
                                tag="b1")]

            # ---- phase 0: XW1 ----
            xt_sb = xtp.tile([P, 4 * NLOC], bf16)
            for k in range(4):
                nc.sync.dma_start(xt_sb[:, k * NLOC:(k + 1) * NLOC],
                                  xt_d[k * P:(k + 1) * P, :])
            for t in range(TILES):
                ps = xwps.tile([P, D], f32, tag="xw")
                for k in range(4):
                    nc.tensor.matmul(
                        out=ps[:],
                        lhsT=xt_sb[:, k * NLOC + t * P: k * NLOC + (t + 1) * P],
                        rhs=w1_sb[:, k * D:(k + 1) * D],
                        start=(k == 0), stop=(k == 3),
                    )
                nc.scalar.activation(out=t_all[0][:, t * P:(t + 1) * P],
                                     in_=ps[:],
                                     func=mybir.ActivationFunctionType.Copy)
                nc.sync.dma_start(t_loc[0][t * P:(t + 1) * P, :],
                                  t_all[0][:, t * P:(t + 1) * P])

            def allgather(li):
                nc.gpsimd.collective_compute(
                    "AllGather",
                    mybir.AluOpType.bypass,
                    replica_groups=[list(range(NCORES))],
                    ins=[t_loc[li][0:ROWS_A, :].opt()],
                    outs=[t_fullA[li][:, :].opt()],
                )
                nc.gpsimd.collective_compute(
                    "AllGather",
                    mybir.AluOpType.bypass,
                    replica_groups=[list(range(NCORES))],
                    ins=[t_loc[li][ROWS_A:NLOC, :].opt()],
                    outs=[t_fullB[li][:, :].opt()],
                )

            def gather_half(li, pp, nt, half):
                tag = "glo" if half == 0 else "ghi"
                nbuf = DEPTH + 1 if half == 0 else HDEPTH + 2
                cap = LO_CAP if half == 0 else HI_CAP
                nch = CLO if half == 0 else CHI
                g = gp.tile([P, 2 * nch * D], bf16, tag=tag, bufs=nbuf,
                            name=f"g{tag}{li}_{pp}")
                ni = nt * cap
                src = (t_fullA[li][:, :] if half == 0
                       else t_fullB[li][:, :])
                c0 = pp * (LO_COLS + HI_COLS) + (0 if half == 0 else LO_COLS)
                gi = nc.gpsimd.dma_gather(
                    g[:, :nt * nch * D].rearrange("p (c d) -> p c d", d=D),
                    src,
                    gidx_sb[:, c0:c0 + ni // 16],
                    ni, ni, D, single_packet=False,
                    queue_num=qctr[0] % 4,
                )
                qctr[0] += 1
                if first_gather[0]:
                    add_dep_helper(gi.ins, lib.ins, reason="lib before gather")
                    first_gather[0] = False
                return g

            def build_s(col):
                s = sp.tile([P, P], bf16, tag="s")
                nc.vector.tensor_scalar(
                    out=s[:], in0=iota_sb[:],
                    scalar1=dpos_sb[:, col:col + 1],
                    scalar2=sval_sb[:, col:col + 1],
                    op0=mybir.AluOpType.is_equal,
                    op1=mybir.AluOpType.mult,
                )
                return s

            def agg_tile(li, qq, ti, g_lo, g_hi):
                t = 2 * qq + ti
                base = qq * 2 * CPT
                agg = aggps.tile([P, P], f32, tag="agg")
                for c in range(CLO):
                    s = build_s(base + ti * CLO + c)
                    nc.tensor.matmul(
                        out=agg[:],
                        lhsT=g_lo[:, (ti * CLO + c) * D:(ti * CLO + c + 1) * D],
                        rhs=s[:], start=(c == 0), stop=False,
                    )
                for c in range(CHI):
                    s = build_s(base + 2 * CLO + ti * CHI + c)
                    nc.tensor.matmul(
                        out=agg[:],
                        lhsT=g_hi[:, (ti * CHI + c) * D:(ti * CHI + c + 1) * D],
                        rhs=s[:], start=False, stop=False,
                    )
                s = build_s(PAIRS * 2 * CPT + t)
                nc.tensor.matmul(
                    out=agg[:],
                    lhsT=t_all[li][:, t * P:(t + 1) * P],
                    rhs=s[:], start=False, stop=True,
                )
                return agg

            # ---- phase 1+2: layer-1 aggregation + XW2 ----
            allgather(0)
            glo_buf = {}
            ghi_buf = {}
            for pp in range(PAIRS + DEPTH):
                nt_pp = 2 if pp < PAIRS and 2 * pp + 1 < TILES else 1
                if pp < PAIRS:
                    glo_buf[pp] = (gather_half(0, pp, nt_pp, 0), nt_pp)
                hh = pp - DEPTH + HDEPTH
                if 0 <= hh < PAIRS:
                    nt_hh = 2 if 2 * hh + 1 < TILES else 1
                    ghi_buf[hh] = gather_half(0, hh, nt_hh, 1)
                qq = pp - DEPTH
                if qq < 0:
                    continue
                g_lo, nt = glo_buf.pop(qq)
                g_hi = ghi_buf.pop(qq)
                for ti in range(nt):
                    t = 2 * qq + ti
                    agg = agg_tile(0, qq, ti, g_lo, g_hi)
                    h1t = hp.tile([P, P], bf16, tag="h")
                    nc.scalar.activation(out=h1t[:], in_=agg[:],
                                         func=mybir.ActivationFunctionType.Relu,
                                         bias=b1_sb[:])
                    ps2 = xwps.tile([P, D], f32, tag="xw2")
                    nc.tensor.matmul(out=ps2[:], lhsT=h1t[:], rhs=w2_sb[:],
                                     start=True, stop=True)
                    nc.vector.tensor_copy(out=t_all[1][:, t * P:(t + 1) * P],
                                          in_=ps2[:])
                    nc.sync.dma_start(t_loc[1][t * P:(t + 1) * P, :],
                                      t_all[1][:, t * P:(t + 1) * P])

            # ---- phase 3+4: layer-2 aggregation + head ----
            allgather(1)
            glo_buf = {}
            ghi_buf = {}
            for pp in range(PAIRS + DEPTH):
                nt_pp = 2 if pp < PAIRS and 2 * pp + 1 < TILES else 1
                if pp < PAIRS:
                    glo_buf[pp] = (gather_half(1, pp, nt_pp, 0), nt_pp)
                hh = pp - DEPTH + HDEPTH
                if 0 <= hh < PAIRS:
                    nt_hh = 2 if 2 * hh + 1 < TILES else 1
                    ghi_buf[hh] = gather_half(1, hh, nt_hh, 1)
                qq = pp - DEPTH
                if qq < 0:
                    continue
                g_lo, nt = glo_buf.pop(qq)
                g_hi = ghi_buf.pop(qq)
                for ti in range(nt):
                    t = 2 * qq + ti
                    agg = agg_tile(1, qq, ti, g_lo, g_hi)
                    h2t = hp.tile([P, P], bf16, tag="h")
                    nc.scalar.activation(out=h2t[:], in_=agg[:],
                                         func=mybir.ActivationFunctionType.Relu,
                                         bias=b2_sb[:])
                    lg = lgps.tile([P, NCLS], f32, tag="lg")
                    nc.tensor.matmul(out=lg[:], lhsT=h2t[:], rhs=wl_sb[:],
                                     start=True, stop=True)
                    l_sb = hdp.tile([P, NCLS], f32, tag="l")
                    nc.vector.tensor_add(out=l_sb[:], in0=lg[:], in1=bl_sb[:])
                    nmx = hdp.tile([P, 1], f32, tag="nmx")
                    nc.vector.reduce_max(out=nmx[:], in_=l_sb[:],
                                         axis=mybir.AxisListType.X, negate=True)
                    e_sb = hdp.tile([P, NCLS], f32, tag="e")
                    nc.scalar.activation(out=e_sb[:], in_=l_sb[:],
                                         func=mybir.ActivationFunctionType.Exp,
                                         bias=nmx[:])
                    sm = hdp.tile([P, 1], f32, tag="sm")
                    nc.vector.reduce_sum(out=sm[:], in_=e_sb[:],
                                         axis=mybir.AxisListType.X)
                    rs = hdp.tile([P, 1], f32, tag="rs")
                    nc.vector.reciprocal(out=rs[:], in_=sm[:])
                    pr = hdp.tile([P, NCLS], f32, tag="pr")
                    nc.scalar.activation(out=pr[:], in_=e_sb[:],
                                         func=mybir.ActivationFunctionType.Copy,
                                         scale=rs[:])
                    nc.sync.dma_start(out_d[t * P:(t + 1) * P, :], pr[:])

    nc.compile()
    return nc


def _preprocess(x, edge_index, W1, b1, W2, b2, Wlin, blin):
    """Host-side graph preprocessing -> per-core input dicts + slot maps."""
    x = np.asarray(x, np.float32)
    ei = np.asarray(edge_index)
    row = ei[0].astype(np.int64)
    col = ei[1].astype(np.int64)

    deg = np.bincount(col, minlength=N).astype(np.float32) + 2.0
    dis = 1.0 / np.sqrt(deg)
    norm_e = (dis[row] * dis[col]).astype(np.float32)
    selfval = (2.0 * dis * dis).astype(np.float32)

    indeg = np.bincount(col, minlength=N)  # per-node in-edges (no self)

    # balanced node->bin assignment (bins = core*TILES + tile), snake by degree
    NB = NCORES * TILES
    order = np.argsort(-indeg, kind="stable")
    bin_of_node = np.empty(N, np.int64)
    full_rounds = N // NB
    rem = N - full_rounds * NB
    fwd = np.arange(NB)
    bwd = fwd[::-1]
    seq = []
    for r in range(full_rounds):
        seq.append(fwd if r % 2 == 0 else bwd)
    if rem:
        seq.append((fwd if full_rounds % 2 == 0 else bwd)[:rem])
    seq = np.concatenate(seq)
    bin_of_node[order] = seq

    # within each pair of tiles, put the tile with more in-edges first
    bin_edges = np.bincount(bin_of_node[col], minlength=NB)
    perm = np.arange(NB)
    for c in range(NCORES):
        for pp in range(TILES // 2):
            b0 = c * TILES + 2 * pp
            if bin_edges[b0] < bin_edges[b0 + 1]:
                perm[b0], perm[b0 + 1] = b0 + 1, b0
    inv = np.empty(NB, np.int64)
    inv[perm] = np.arange(NB)
    bin_of_node = inv[bin_of_node]

    pos_in_bin = np.empty(N, np.int64)
    srt = np.argsort(bin_of_node, kind="stable")
    cnt = np.bincount(bin_of_node, minlength=NB)
    assert cnt.max() <= P
    starts = np.zeros(NB + 1, np.int64)
    np.cumsum(cnt, out=starts[1:])
    pos_in_bin[srt] = np.arange(N) - starts[bin_of_node[srt]]

    bin_edge_cnt = np.bincount(bin_of_node[col], minlength=NB)
    assert bin_edge_cnt.max() <= ECAP, (
        f"bin edge overflow: {bin_edge_cnt.max()} > {ECAP}")

    core_of_node = bin_of_node // TILES
    tile_of_node = bin_of_node % TILES
    # table row: AG-A region holds tiles 0..TILES_A-1 of every core, then AG-B
    gslot = np.where(
        tile_of_node < TILES_A,
        core_of_node * ROWS_A + tile_of_node * P + pos_in_bin,
        LO_LIM + core_of_node * ROWS_B + (tile_of_node - TILES_A) * P + pos_in_bin,
    )

    # per-edge: destination bin + dest position; source table slot
    e_bin = bin_of_node[col]
    e_dpos = pos_in_bin[col]
    e_src = gslot[row]

    # group edges by bin
    e_order = np.argsort(e_bin, kind="stable")
    eb = e_bin[e_order]
    ed = e_dpos[e_order]
    es = e_src[e_order]
    en = norm_e[e_order]
    bstarts = np.searchsorted(eb, np.arange(NB + 1))

    in_maps = []
    for c in range(NCORES):
        gidx = np.zeros((P, PAIRS * (LO_COLS + HI_COLS)), np.int16)
        dpos = np.zeros((P, SCOLS), np.float32)
        sval = np.zeros((P, SCOLS), np.float32)
        for pp in range(PAIRS):
            nt = 2 if 2 * pp + 1 < TILES else 1
            # per tile: (rel_idx, dpos, norm) for lo and hi halves
            halves = {0: [], 1: []}
            for ti in range(nt):
                t = 2 * pp + ti
                b = c * TILES + t
                lo_f, hi_f = bstarts[b], bstarts[b + 1]
                srcs = es[lo_f:hi_f]
                dposs = ed[lo_f:hi_f]
                nrm = en[lo_f:hi_f]
                ne = len(srcs)
                is_lo = srcs < LO_LIM
                lo_n = int(is_lo.sum())
                assert lo_n <= LO_CAP and ne - lo_n <= HI_CAP, (c, t, ne, lo_n)
                for half in (0, 1):
                    sel = is_lo if half == 0 else ~is_lo
                    hs, hd, hn = srcs[sel], dposs[sel], nrm[sel]
                    rel = hs if half == 0 else hs - LO_LIM
                    o3 = np.argsort(rel, kind="stable")
                    halves[half].append((rel[o3], hd[o3], hn[o3]))
            for half in (0, 1):
                cap = LO_CAP if half == 0 else HI_CAP
                ncap = CLO if half == 0 else CHI
                flat = np.zeros(nt * cap, np.int64)
                for ti in range(nt):
                    rel, hd, hn = halves[half][ti]
                    k = len(rel)
                    flat[ti * cap: ti * cap + k] = rel
                    if k < cap:
                        # padding: repeat last valid idx (or 0); NO -1 trim
                        flat[ti * cap + k: (ti + 1) * cap] = rel[-1] if k else 0
                    # S-value columns for this tile's chunks in this half
                    base = pp * 2 * CPT + (ti * CLO if half == 0
                                           else 2 * CLO + ti * CHI)
                    ii = np.arange(k)
                    cidx = base + ii // P
                    pidx = ii % P
                    dpos[pidx, cidx] = hd.astype(np.float32)
                    sval[pidx, cidx] = hn
                w = flat.reshape(len(flat) // 16, 16).T.astype(np.int16)
                c0 = pp * (LO_COLS + HI_COLS) + (0 if half == 0 else LO_COLS)
                gidx[:, c0:c0 + len(flat) // 16] = np.tile(w, (8, 1))
        # self-loop diagonal columns
        mine = np.where(core_of_node == c)[0]
        lslot = tile_of_node[mine] * P + pos_in_bin[mine]
        for t in range(TILES):
            colx = PAIRS * 2 * CPT + t
            dpos[:, colx] = np.arange(P, dtype=np.float32)
            sel = tile_of_node[mine] == t
            nodes_t = mine[sel]
            pos_t = pos_in_bin[nodes_t]
            v = np.zeros(P, np.float32)
            v[pos_t] = selfval[nodes_t]
            sval[:, colx] = v
        # x slice, transposed, padded
        xt = np.zeros((FIN, NLOC), ml_dtypes.bfloat16)
        xt[:, lslot] = x[mine].T.astype(ml_dtypes.bfloat16)
        iota = np.tile(np.arange(P, dtype=np.float32)[None, :], (P, 1))
        in_maps.append({
            "xt": xt,
            "w1": np.asarray(W1).astype(ml_dtypes.bfloat16),
            "w2": np.asarray(W2).astype(ml_dtypes.bfloat16),
            "wl": np.asarray(Wlin).astype(ml_dtypes.bfloat16),
            "b1": np.asarray(b1, np.float32).reshape(P, 1),
            "b2": np.asarray(b2, np.float32).reshape(P, 1),
            "bl": np.tile(np.asarray(blin, np.float32).reshape(1, NCLS), (P, 1)),
            "gidx": gidx,
            "iota": iota.astype(ml_dtypes.bfloat16),
            "dpos": dpos,
            "sval": sval,
        })
    return in_maps, core_of_node, tile_of_node, pos_in_bin


def kernel(x, edge_index, W1, b1, W2, b2, Wlin, blin):
    global _PROGRAM, LAST_EXEC_NS, LAST_RESULT
    in_maps, core_of, tile_of, pos_of = _preprocess(
        x, edge_index, W1, b1, W2, b2, Wlin, blin)
    if _PROGRAM is None:
        _PROGRAM = _build_program()
    res = run_bass_kernel_spmd(
        _PROGRAM, in_maps, core_ids=list(range(NCORES)), trace=TRACE)
    LAST_EXEC_NS = res.exec_time_ns
    LAST_RESULT = res
    out = np.empty((N, NCLS), np.float32)
    per_core = [res.results[c]["probs"] for c in range(NCORES)]
    lslot = tile_of * P + pos_of
    for c in range(NCORES):
        mine = np.where(core_of == c)[0]
        out[mine] = per_core[c][lslot[mine]]
    return out


# revision 13
# speedup vs baseline: 1.6089x; 1.6089x over previous
"""2-layer GCN (improved=True) + linear head + softmax on 8 Trainium2 cores.

Strategy (dest-node partitioning):
- Nodes assigned to 8 cores x 49 tiles x 128 slots via balanced bin-packing on
  in-degree (max 2176 in-edges per tile; self-loops are NOT gathered).
- Per layer: each core computes XW for its slots (node-major [n,128]),
  AllGather replicates the table to every core's HBM (Shared scratch for the
  fast collective path), then per dest-tile-pair the core gathers source rows
  with dma_gather (lo window 2048 idx / hi window 2304 idx, trailing -1
  padding is trimmed by the ucode) and scatter-adds them with one-hot matmuls
  agg^T[d, n] += G_chunk^T[d, e] @ S_chunk[e, n].
- S chunks are built ON-CHIP by the vector engine from compact per-chunk
  (dest-pos, norm) columns: S[e, n] = (iota[n] == dpos[e]) * val[e]; no dense
  S matrices ever touch HBM.
- The self-loop term (norm 2*dis^2) is one extra matmul per tile with
  lhsT = the tile's own XW output (kept in SBUF) and a diagonal S built the
  same way (dpos = arange, val = 2*dis^2).
- Gather calls rotate across all 4 SWDGE queues so descriptor generation for
  different calls can overlap on different GPSIMD core pairs.
- Head: logits = H2 @ Wlin + blin, softmax over 8 classes, on-chip.

kernel() is self-contained: host-side numpy does all graph preprocessing;
the device program is identical on all 8 cores, only data differs.
"""
import sys

sys.path.insert(0, "/opt/trn_rl_repo")

import numpy as np
import ml_dtypes

import concourse.bass as bass
import concourse.bacc as bacc
import concourse.mybir as mybir
import concourse.tile as tile
from concourse.tile_rust import add_dep_helper
from concourse.bass_utils import run_bass_kernel_spmd
from concourse.library_config import mlp

# problem constants
N = 50000
E = 800000
FIN = 512
D = 128
NCLS = 8
NCORES = 8

# sharding constants
P = 128
TILES = 49
NLOC = TILES * P            # 6272 slots per core
VTOT = NCORES * NLOC        # 50176 table rows
LO_CAP = 1152               # per-tile lo-window edge cap (9 chunks)
HI_CAP = 1152               # per-tile hi-window edge cap (9 chunks)
ECAP = LO_CAP + HI_CAP      # 2304 in-edges per tile
CLO = LO_CAP // P           # 9 lo chunks per tile
CHI = HI_CAP // P           # 9 hi chunks per tile
CPT = CLO + CHI             # 18 edge chunks per tile
TILES_A = 25                # tiles in AG phase A
ROWS_A = TILES_A * P        # 3328 rows/core in phase A
LO_LIM = NCORES * ROWS_A    # 26624: lo gathers read only the AG-A region
TILES_B = TILES - TILES_A
ROWS_B = TILES_B * P
PAIRS = (TILES + 1) // 2
DEPTH = 8                   # lo-gather software-pipeline depth (pairs)
HDEPTH = 2                  # hi-gather prefetch depth (pairs)
LO_COLS = 2 * LO_CAP // 16  # 128 int16 idx cols per pair (lo)
HI_COLS = 2 * HI_CAP // 16  # 144 int16 idx cols per pair (hi)
SCOLS = PAIRS * (2 * CPT) + TILES  # dpos/val columns (incl self diag)

TRACE = False
LAST_EXEC_NS = None
LAST_RESULT = None

_PROGRAM = None


def _build_program():
    nc = bacc.Bacc(None, target_bir_lowering=False, num_swdge_queues=4)
    f32 = mybir.dt.float32
    bf16 = mybir.dt.bfloat16

    xt_d = nc.dram_tensor("xt", [FIN, NLOC], bf16, kind="ExternalInput")
    w1_d = nc.dram_tensor("w1", [FIN, D], bf16, kind="ExternalInput")
    w2_d = nc.dram_tensor("w2", [D, D], bf16, kind="ExternalInput")
    wl_d = nc.dram_tensor("wl", [D, NCLS], bf16, kind="ExternalInput")
    b1_d = nc.dram_tensor("b1", [P, 1], f32, kind="ExternalInput")
    b2_d = nc.dram_tensor("b2", [P, 1], f32, kind="ExternalInput")
    bl_d = nc.dram_tensor("bl", [P, NCLS], f32, kind="ExternalInput")
    gidx_d = nc.dram_tensor("gidx", [P, PAIRS * (LO_COLS + HI_COLS)],
                            mybir.dt.int16, kind="ExternalInput")
    sval_d = nc.dram_tensor("sval", [TILES, P, (CPT + 1) * P], bf16,
                            kind="ExternalInput")
    cnt_d = nc.dram_tensor("cnt", [P, 2 * PAIRS], mybir.dt.int32,
                           kind="ExternalInput")
    out_d = nc.dram_tensor("probs", [NLOC, NCLS], f32, kind="ExternalOutput")

    with tile.TileContext(nc) as tc:
        lib = nc.gpsimd.load_library(mlp)
        first_gather = [True]
        qctr = [0]
        cnt_regs = [nc.gpsimd.alloc_register(f"cntreg{i}") for i in range(4)]

        with (
            tc.tile_pool(name="const", bufs=1) as cp,
            tc.tile_pool(name="xtp", bufs=1) as xtp,
            tc.tile_pool(name="gpool", bufs=4) as gp,
            tc.tile_pool(name="spool", bufs=8) as sp,
            tc.tile_pool(name="hpool", bufs=3) as hp,
            tc.tile_pool(name="headp", bufs=3) as hdp,
            tc.tile_pool(name="xwps", bufs=2, space="PSUM") as xwps,
            tc.tile_pool(name="aggps", bufs=2, space="PSUM") as aggps,
            tc.tile_pool(name="lgps", bufs=2, space="PSUM") as lgps,
            tc.tile_pool(name="dram1", bufs=1, space="DRAM") as dr1,
            tc.tile_pool(name="dram2", bufs=1, space="DRAM") as dr2,
            tc.tile_pool(name="dram3", bufs=1, space="DRAM") as dr3,
            tc.tile_pool(name="dram4", bufs=1, space="DRAM") as dr4,
        ):
            # ---- constants to SBUF ----
            w1_sb = cp.tile([P, 4 * D], bf16)
            for k in range(4):
                nc.sync.dma_start(w1_sb[:, k * D:(k + 1) * D],
                                  w1_d[k * P:(k + 1) * P, :])
            w2_sb = cp.tile([P, D], bf16)
            nc.sync.dma_start(w2_sb[:], w2_d[:])
            wl_sb = cp.tile([P, NCLS], bf16)
            nc.sync.dma_start(wl_sb[:], wl_d[:])
            b1_sb = cp.tile([P, 1], f32)
            nc.sync.dma_start(b1_sb[:], b1_d[:])
            b2_sb = cp.tile([P, 1], f32)
            nc.sync.dma_start(b2_sb[:], b2_d[:])
            bl_sb = cp.tile([P, NCLS], f32)
            nc.sync.dma_start(bl_sb[:], bl_d[:])
            gidx_sb = cp.tile([P, PAIRS * (LO_COLS + HI_COLS)], mybir.dt.int16)
            nc.sync.dma_start(gidx_sb[:], gidx_d[:])
            cnt_sb = cp.tile([P, 2 * PAIRS], mybir.dt.int32)
            nc.sync.dma_start(cnt_sb[:], cnt_d[:])
            # per-layer local XW tables kept in SBUF for the self-loop matmul
            t_all = [cp.tile([P, NLOC], bf16, name="t_all0"),
                     cp.tile([P, NLOC], bf16, name="t_all1")]

            t_loc = [dr1.tile([NLOC, D], bf16, name="t_loc0"),
                     dr2.tile([NLOC, D], bf16, name="t_loc1")]
            t_fullA = [dr3.tile([LO_LIM, D], bf16, name="t_fullA0", tag="a0"),
                       dr3.tile([LO_LIM, D], bf16, name="t_fullA1", tag="a1")]
            t_fullB = [dr4.tile([VTOT - LO_LIM, D], bf16, name="t_fullB0",
                                tag="b0"),
                       dr4.tile([VTOT - LO_LIM, D], bf16, name="t_fullB1",
                                tag="b1")]

            # zero all gather buffers once so count-trimmed (ungathered)
            # tail positions hold finite values (NaN*0 = NaN in the PE)
            for i in range(DEPTH + 1):
                gz = gp.tile([P, 2 * CLO * D], bf16, tag="glo", bufs=DEPTH + 1,
                             name=f"gz_lo{i}")
                nc.vector.memset(gz[:], 0)
            for i in range(HDEPTH + 2):
                gz = gp.tile([P, 2 * CHI * D], bf16, tag="ghi", bufs=HDEPTH + 2,
                             name=f"gz_hi{i}")
                nc.vector.memset(gz[:], 0)

            # ---- phase 0: XW1 ----
            xt_sb = xtp.tile([P, 4 * NLOC], bf16)
            for k in range(4):
                nc.sync.dma_start(xt_sb[:, k * NLOC:(k + 1) * NLOC],
                                  xt_d[k * P:(k + 1) * P, :])
            for t in range(TILES):
                ps = xwps.tile([P, D], f32, tag="xw")
                for k in range(4):
                    nc.tensor.matmul(
                        out=ps[:],
                        lhsT=xt_sb[:, k * NLOC + t * P: k * NLOC + (t + 1) * P],
                        rhs=w1_sb[:, k * D:(k + 1) * D],
                        start=(k == 0), stop=(k == 3),
                    )
                nc.scalar.activation(out=t_all[0][:, t * P:(t + 1) * P],
                                     in_=ps[:],
                                     func=mybir.ActivationFunctionType.Copy)
                nc.sync.dma_start(t_loc[0][t * P:(t + 1) * P, :],
                                  t_all[0][:, t * P:(t + 1) * P])

            def allgather(li):
                nc.gpsimd.collective_compute(
                    "AllGather",
                    mybir.AluOpType.bypass,
                    replica_groups=[list(range(NCORES))],
                    ins=[t_loc[li][0:ROWS_A, :].opt()],
                    outs=[t_fullA[li][:, :].opt()],
                )
                nc.gpsimd.collective_compute(
                    "AllGather",
                    mybir.AluOpType.bypass,
                    replica_groups=[list(range(NCORES))],
                    ins=[t_loc[li][ROWS_A:NLOC, :].opt()],
                    outs=[t_fullB[li][:, :].opt()],
                )

            def gather_half(li, pp, nt, half):
                tag = "glo" if half == 0 else "ghi"
                nbuf = DEPTH + 1 if half == 0 else HDEPTH + 2
                cap = LO_CAP if half == 0 else HI_CAP
                nch = CLO if half == 0 else CHI
                g = gp.tile([P, 2 * nch * D], bf16, tag=tag, bufs=nbuf,
                            name=f"g{tag}{li}_{pp}")
                ni = nt * cap
                src = (t_fullA[li][:, :] if half == 0
                       else t_fullB[li][:, :])
                c0 = pp * (LO_COLS + HI_COLS) + (0 if half == 0 else LO_COLS)
                qn = qctr[0] % 4
                ni_reg = cnt_regs[qn]
                nc.gpsimd.reg_load(ni_reg,
                                   cnt_sb[0:1, 2 * pp + half:2 * pp + half + 1])
                gi = nc.gpsimd.dma_gather(
                    g[:, :nt * nch * D].rearrange("p (c d) -> p c d", d=D),
                    src,
                    gidx_sb[:, c0:c0 + ni // 16],
                    ni, ni_reg, D, single_packet=False,
                    queue_num=qn,
                )
                qctr[0] += 1
                if first_gather[0]:
                    add_dep_helper(gi.ins, lib.ins, reason="lib before gather")
                    first_gather[0] = False
                return g

            def agg_tile(li, qq, ti, g_lo, g_hi):
                t = 2 * qq + ti
                s_sb = sp.tile([P, (CPT + 1) * P], bf16, tag="s", bufs=5)
                nc.sync.dma_start(s_sb[:], sval_d[t, :, :])
                agg = aggps.tile([P, P], f32, tag="agg")
                for c in range(CLO):
                    nc.tensor.matmul(
                        out=agg[:],
                        lhsT=g_lo[:, (ti * CLO + c) * D:(ti * CLO + c + 1) * D],
                        rhs=s_sb[:, c * P:(c + 1) * P],
                        start=(c == 0), stop=False,
                    )
                for c in range(CHI):
                    nc.tensor.matmul(
                        out=agg[:],
                        lhsT=g_hi[:, (ti * CHI + c) * D:(ti * CHI + c + 1) * D],
                        rhs=s_sb[:, (CLO + c) * P:(CLO + c + 1) * P],
                        start=False, stop=False,
                    )
                nc.tensor.matmul(
                    out=agg[:],
                    lhsT=t_all[li][:, t * P:(t + 1) * P],
                    rhs=s_sb[:, CPT * P:(CPT + 1) * P],
                    start=False, stop=True,
                )
                return agg

            # ---- phase 1+2: layer-1 aggregation + XW2 ----
            allgather(0)
            glo_buf = {}
            ghi_buf = {}
            for pp in range(PAIRS + DEPTH):
                nt_pp = 2 if pp < PAIRS and 2 * pp + 1 < TILES else 1
                if pp < PAIRS:
                    glo_buf[pp] = (gather_half(0, pp, nt_pp, 0), nt_pp)
                hh = pp - DEPTH + HDEPTH
                if 0 <= hh < PAIRS:
                    nt_hh = 2 if 2 * hh + 1 < TILES else 1
                    ghi_buf[hh] = gather_half(0, hh, nt_hh, 1)
                qq = pp - DEPTH
                if qq < 0:
                    continue
                g_lo, nt = glo_buf.pop(qq)
                g_hi = ghi_buf.pop(qq)
                for ti in range(nt):
                    t = 2 * qq + ti
                    agg = agg_tile(0, qq, ti, g_lo, g_hi)
                    h1t = hp.tile([P, P], bf16, tag="h")
                    nc.scalar.activation(out=h1t[:], in_=agg[:],
                                         func=mybir.ActivationFunctionType.Relu,
                                         bias=b1_sb[:])
                    ps2 = xwps.tile([P, D], f32, tag="xw2")
                    nc.tensor.matmul(out=ps2[:], lhsT=h1t[:], rhs=w2_sb[:],
                                     start=True, stop=True)
                    nc.vector.tensor_copy(out=t_all[1][:, t * P:(t + 1) * P],
                                          in_=ps2[:])
                    nc.sync.dma_start(t_loc[1][t * P:(t + 1) * P, :],
                                      t_all[1][:, t * P:(t + 1) * P])

            # ---- phase 3+4: layer-2 aggregation + head ----
            allgather(1)
            glo_buf = {}
            ghi_buf = {}
            for pp in range(PAIRS + DEPTH):
                nt_pp = 2 if pp < PAIRS and 2 * pp + 1 < TILES else 1
                if pp < PAIRS:
                    glo_buf[pp] = (gather_half(1, pp, nt_pp, 0), nt_pp)
                hh = pp - DEPTH + HDEPTH
                if 0 <= hh < PAIRS:
                    nt_hh = 2 if 2 * hh + 1 < TILES else 1
                    ghi_buf[hh] = gather_half(1, hh, nt_hh, 1)
                qq = pp - DEPTH
                if qq < 0:
                    continue
                g_lo, nt = glo_buf.pop(qq)
                g_hi = ghi_buf.pop(qq)
                for ti in range(nt):
                    t = 2 * qq + ti
                    agg = agg_tile(1, qq, ti, g_lo, g_hi)
                    h2t = hp.tile([P, P], bf16, tag="h")
                    nc.scalar.activation(out=h2t[:], in_=agg[:],
                                         func=mybir.ActivationFunctionType.Relu,
                                         bias=b2_sb[:])
                    lg = lgps.tile([P, NCLS], f32, tag="lg")
                    nc.tensor.matmul(out=lg[:], lhsT=h2t[:], rhs=wl_sb[:],
                                     start=True, stop=True)
                    l_sb = hdp.tile([P, NCLS], f32, tag="l")
                    nc.vector.tensor_add(out=l_sb[:], in0=lg[:], in1=bl_sb[:])
                    nmx = hdp.tile([P, 1], f32, tag="nmx")
                    nc.vector.reduce_max(out=nmx[:], in_=l_sb[:],
                                         axis=mybir.AxisListType.X, negate=True)
                    e_sb = hdp.tile([P, NCLS], f32, tag="e")
                    nc.scalar.activation(out=e_sb[:], in_=l_sb[:],
                                         func=mybir.ActivationFunctionType.Exp,
                                         bias=nmx[:])
                    sm = hdp.tile([P, 1], f32, tag="sm")
                    nc.vector.reduce_sum(out=sm[:], in_=e_sb[:],
                                         axis=mybir.AxisListType.X)
                    rs = hdp.tile([P, 1], f32, tag="rs")
                    nc.vector.reciprocal(out=rs[:], in_=sm[:])
                    pr = hdp.tile([P, NCLS], f32, tag="pr")
                    nc.scalar.activation(out=pr[:], in_=e_sb[:],
                                         func=mybir.ActivationFunctionType.Copy,
                                         scale=rs[:])
                    nc.sync.dma_start(out_d[t * P:(t + 1) * P, :], pr[:])

    nc.compile()
    return nc


def _preprocess(x, edge_index, W1, b1, W2, b2, Wlin, blin):
    """Host-side graph preprocessing -> per-core input dicts + slot maps."""
    x = np.asarray(x, np.float32)
    ei = np.asarray(edge_index)
    row = ei[0].astype(np.int64)
    col = ei[1].astype(np.int64)

    deg = np.bincount(col, minlength=N).astype(np.float32) + 2.0
    dis = 1.0 / np.sqrt(deg)
    norm_e = (dis[row] * dis[col]).astype(np.float32)
    selfval = (2.0 * dis * dis).astype(np.float32)

    indeg = np.bincount(col, minlength=N)  # per-node in-edges (no self)

    # balanced node->bin assignment (bins = core*TILES + tile), snake by degree
    NB = NCORES * TILES
    order = np.argsort(-indeg, kind="stable")
    bin_of_node = np.empty(N, np.int64)
    full_rounds = N // NB
    rem = N - full_rounds * NB
    fwd = np.arange(NB)
    bwd = fwd[::-1]
    seq = []
    for r in range(full_rounds):
        seq.append(fwd if r % 2 == 0 else bwd)
    if rem:
        seq.append((fwd if full_rounds % 2 == 0 else bwd)[:rem])
    seq = np.concatenate(seq)
    bin_of_node[order] = seq

    # within each pair of tiles, put the tile with more in-edges first
    bin_edges = np.bincount(bin_of_node[col], minlength=NB)
    perm = np.arange(NB)
    for c in range(NCORES):
        for pp in range(TILES // 2):
            b0 = c * TILES + 2 * pp
            if bin_edges[b0] < bin_edges[b0 + 1]:
                perm[b0], perm[b0 + 1] = b0 + 1, b0
    inv = np.empty(NB, np.int64)
    inv[perm] = np.arange(NB)
    bin_of_node = inv[bin_of_node]

    pos_in_bin = np.empty(N, np.int64)
    srt = np.argsort(bin_of_node, kind="stable")
    cnt = np.bincount(bin_of_node, minlength=NB)
    assert cnt.max() <= P
    starts = np.zeros(NB + 1, np.int64)
    np.cumsum(cnt, out=starts[1:])
    pos_in_bin[srt] = np.arange(N) - starts[bin_of_node[srt]]

    bin_edge_cnt = np.bincount(bin_of_node[col], minlength=NB)
    assert bin_edge_cnt.max() <= ECAP, (
        f"bin edge overflow: {bin_edge_cnt.max()} > {ECAP}")

    core_of_node = bin_of_node // TILES
    tile_of_node = bin_of_node % TILES
    # table row: AG-A region holds tiles 0..TILES_A-1 of every core, then AG-B
    gslot = np.where(
        tile_of_node < TILES_A,
        core_of_node * ROWS_A + tile_of_node * P + pos_in_bin,
        LO_LIM + core_of_node * ROWS_B + (tile_of_node - TILES_A) * P + pos_in_bin,
    )

    # per-edge: destination bin + dest position; source table slot
    e_bin = bin_of_node[col]
    e_dpos = pos_in_bin[col]
    e_src = gslot[row]

    # group edges by bin
    e_order = np.argsort(e_bin, kind="stable")
    eb = e_bin[e_order]
    ed = e_dpos[e_order]
    es = e_src[e_order]
    en = norm_e[e_order]
    bstarts = np.searchsorted(eb, np.arange(NB + 1))

    in_maps = []
    for c in range(NCORES):
        gidx = np.zeros((P, PAIRS * (LO_COLS + HI_COLS)), np.int16)
        sval = np.zeros((TILES, P, (CPT + 1) * P), ml_dtypes.bfloat16)
        cntv = np.ones((2 * PAIRS,), np.int32)
        for pp in range(PAIRS):
            nt = 2 if 2 * pp + 1 < TILES else 1
            # per tile: (rel_idx, dpos, norm) for lo and hi halves
            halves = {0: [], 1: []}
            for ti in range(nt):
                t = 2 * pp + ti
                b = c * TILES + t
                lo_f, hi_f = bstarts[b], bstarts[b + 1]
                srcs = es[lo_f:hi_f]
                dposs = ed[lo_f:hi_f]
                nrm = en[lo_f:hi_f]
                ne = len(srcs)
                is_lo = srcs < LO_LIM
                lo_n = int(is_lo.sum())
                assert lo_n <= LO_CAP and ne - lo_n <= HI_CAP, (c, t, ne, lo_n)
                for half in (0, 1):
                    sel = is_lo if half == 0 else ~is_lo
                    hs, hd, hn = srcs[sel], dposs[sel], nrm[sel]
                    rel = hs if half == 0 else hs - LO_LIM
                    o3 = np.argsort(rel, kind="stable")
                    halves[half].append((rel[o3], hd[o3], hn[o3]))
            for half in (0, 1):
                cap = LO_CAP if half == 0 else HI_CAP
                flat = np.zeros(nt * cap, np.int64)
                last_k = 0
                for ti in range(nt):
                    t = 2 * pp + ti
                    rel, hd, hn = halves[half][ti]
                    k = len(rel)
                    last_k = k
                    flat[ti * cap: ti * cap + k] = rel
                    if k < cap:
                        # interior padding: repeat last valid idx (or 0)
                        flat[ti * cap + k: (ti + 1) * cap] = rel[-1] if k else 0
                    # dense S values: tile-local position -> (chunk, partition)
                    cbase = 0 if half == 0 else CLO
                    ii = np.arange(k)
                    cidx = cbase + ii // P
                    pidx = ii % P
                    sval[t, pidx, cidx * P + hd] = hn
                cnt_ph = max(1, (nt - 1) * cap + last_k)
                cntv[2 * pp + half] = cnt_ph
                # trailing padding past the real count is -1: the ucode trims
                # it and the decode reserves ring space from the count reg,
                # so both sides agree on the descriptor count
                flat[cnt_ph:] = -1
                w = flat.reshape(len(flat) // 16, 16).T.astype(np.int16)
                c0 = pp * (LO_COLS + HI_COLS) + (0 if half == 0 else LO_COLS)
                gidx[:, c0:c0 + len(flat) // 16] = np.tile(w, (8, 1))
        # self-loop diagonal chunk (chunk CPT of each tile)
        mine = np.where(core_of_node == c)[0]
        lslot = tile_of_node[mine] * P + pos_in_bin[mine]
        for t in range(TILES):
            sel = tile_of_node[mine] == t
            nodes_t = mine[sel]
            pos_t = pos_in_bin[nodes_t]
            sval[t, pos_t, CPT * P + pos_t] = selfval[nodes_t]
        # x slice, transposed, padded
        xt = np.zeros((FIN, NLOC), ml_dtypes.bfloat16)
        xt[:, lslot] = x[mine].T.astype(ml_dtypes.bfloat16)
        in_maps.append({
            "xt": xt,
            "w1": np.asarray(W1).astype(ml_dtypes.bfloat16),
            "w2": np.asarray(W2).astype(ml_dtypes.bfloat16),
            "wl": np.asarray(Wlin).astype(ml_dtypes.bfloat16),
            "b1": np.asarray(b1, np.float32).reshape(P, 1),
            "b2": np.asarray(b2, np.float32).reshape(P, 1),
            "bl": np.tile(np.asarray(blin, np.float32).reshape(1, NCLS), (P, 1)),
            "gidx": gidx,
            "sval": sval,
            "cnt": np.tile(cntv[None, :], (P, 1)),
        })
    return in_maps, core_of_node, tile_of_node, pos_in_bin


def kernel(x, edge_index, W1, b1, W2, b2, Wlin, blin):
    global _PROGRAM, LAST_EXEC_NS, LAST_RESULT
    in_maps, core_of, tile_of, pos_of = _preprocess(
        x, edge_index, W1, b1, W2, b2, Wlin, blin)
    if _PROGRAM is None:
        _PROGRAM = _build_program()
    res = run_bass_kernel_spmd(
        _PROGRAM, in_maps, core_ids=list(range(NCORES)), trace=TRACE)
    LAST_EXEC_NS = res.exec_time_ns
    LAST_RESULT = res
    out = np.empty((N, NCLS), np.float32)
    per_core = [res.results[c]["probs"] for c in range(NCORES)]
    lslot = tile_of * P + pos_of
    for c in range(NCORES):
        mine = np.where(core_of == c)[0]
        out[mine] = per_core[c][lslot[mine]]
    return out


# revision 14
# speedup vs baseline: 1.6635x; 1.0339x over previous
"""2-layer GCN (improved=True) + linear head + softmax on 8 Trainium2 cores.

Strategy (dest-node partitioning):
- Nodes assigned to 8 cores x 49 tiles x 128 slots via balanced bin-packing on
  in-degree (max 2176 in-edges per tile; self-loops are NOT gathered).
- Per layer: each core computes XW for its slots (node-major [n,128]),
  AllGather replicates the table to every core's HBM (Shared scratch for the
  fast collective path), then per dest-tile-pair the core gathers source rows
  with dma_gather (lo window 2048 idx / hi window 2304 idx, trailing -1
  padding is trimmed by the ucode) and scatter-adds them with one-hot matmuls
  agg^T[d, n] += G_chunk^T[d, e] @ S_chunk[e, n].
- S chunks are built ON-CHIP by the vector engine from compact per-chunk
  (dest-pos, norm) columns: S[e, n] = (iota[n] == dpos[e]) * val[e]; no dense
  S matrices ever touch HBM.
- The self-loop term (norm 2*dis^2) is one extra matmul per tile with
  lhsT = the tile's own XW output (kept in SBUF) and a diagonal S built the
  same way (dpos = arange, val = 2*dis^2).
- Gather calls rotate across all 4 SWDGE queues so descriptor generation for
  different calls can overlap on different GPSIMD core pairs.
- Head: logits = H2 @ Wlin + blin, softmax over 8 classes, on-chip.

kernel() is self-contained: host-side numpy does all graph preprocessing;
the device program is identical on all 8 cores, only data differs.
"""
import sys

sys.path.insert(0, "/opt/trn_rl_repo")

import numpy as np
import ml_dtypes

import concourse.bass as bass
import concourse.bacc as bacc
import concourse.mybir as mybir
import concourse.tile as tile
from concourse.tile_rust import add_dep_helper
from concourse.bass_utils import run_bass_kernel_spmd
from concourse.library_config import mlp

# problem constants
N = 50000
E = 800000
FIN = 512
D = 128
NCLS = 8
NCORES = 8

# sharding constants
P = 128
TILES = 49
NLOC = TILES * P            # 6272 slots per core
VTOT = NCORES * NLOC        # 50176 table rows
LO_CAP = 1152               # per-tile lo-window edge cap (9 chunks)
HI_CAP = 1152               # per-tile hi-window edge cap (9 chunks)
ECAP = LO_CAP + HI_CAP      # 2304 in-edges per tile
CLO = LO_CAP // P           # 9 lo chunks per tile
CHI = HI_CAP // P           # 9 hi chunks per tile
CPT = CLO + CHI             # 18 edge chunks per tile
TILES_A = 25                # tiles in AG phase A
ROWS_A = TILES_A * P        # 3328 rows/core in phase A
LO_LIM = NCORES * ROWS_A    # 26624: lo gathers read only the AG-A region
TILES_B = TILES - TILES_A
ROWS_B = TILES_B * P
PAIRS = (TILES + 1) // 2
DEPTH = 8                   # lo-gather software-pipeline depth (pairs)
HDEPTH = 2                  # hi-gather prefetch depth (pairs)
LO_COLS = 2 * LO_CAP // 16  # 128 int16 idx cols per pair (lo)
HI_COLS = 2 * HI_CAP // 16  # 144 int16 idx cols per pair (hi)
SCOLS = PAIRS * (2 * CPT) + TILES  # dpos/val columns (incl self diag)

TRACE = False
LAST_EXEC_NS = None
LAST_RESULT = None

_PROGRAM = None


def _build_program():
    nc = bacc.Bacc(None, target_bir_lowering=False, num_swdge_queues=4)
    f32 = mybir.dt.float32
    bf16 = mybir.dt.bfloat16

    xt_d = nc.dram_tensor("xt", [FIN, NLOC], bf16, kind="ExternalInput")
    w1_d = nc.dram_tensor("w1", [FIN, D], bf16, kind="ExternalInput")
    w2_d = nc.dram_tensor("w2", [D, D], bf16, kind="ExternalInput")
    wl_d = nc.dram_tensor("wl", [D, NCLS], bf16, kind="ExternalInput")
    b1_d = nc.dram_tensor("b1", [P, 1], f32, kind="ExternalInput")
    b2_d = nc.dram_tensor("b2", [P, 1], f32, kind="ExternalInput")
    bl_d = nc.dram_tensor("bl", [P, NCLS], f32, kind="ExternalInput")
    gidx_d = nc.dram_tensor("gidx", [P, PAIRS * (LO_COLS + HI_COLS)],
                            mybir.dt.int16, kind="ExternalInput")
    sval_d = nc.dram_tensor("sval", [TILES, P, (CPT + 1) * P], bf16,
                            kind="ExternalInput")
    cnt_d = nc.dram_tensor("cnt", [P, 2 * PAIRS], mybir.dt.int32,
                           kind="ExternalInput")
    out_d = nc.dram_tensor("probs", [NLOC, NCLS], f32, kind="ExternalOutput")

    with tile.TileContext(nc) as tc:
        lib = nc.gpsimd.load_library(mlp)
        first_gather = [True]
        qctr = [0]
        cnt_regs = [nc.gpsimd.alloc_register(f"cntreg{i}") for i in range(4)]

        with (
            tc.tile_pool(name="const", bufs=1) as cp,
            tc.tile_pool(name="xtp", bufs=1) as xtp,
            tc.tile_pool(name="gpool", bufs=4) as gp,
            tc.tile_pool(name="spool", bufs=8) as sp,
            tc.tile_pool(name="hpool", bufs=3) as hp,
            tc.tile_pool(name="headp", bufs=3) as hdp,
            tc.tile_pool(name="xwps", bufs=2, space="PSUM") as xwps,
            tc.tile_pool(name="aggps", bufs=2, space="PSUM") as aggps,
            tc.tile_pool(name="lgps", bufs=2, space="PSUM") as lgps,
            tc.tile_pool(name="dram1", bufs=1, space="DRAM") as dr1,
            tc.tile_pool(name="dram2", bufs=1, space="DRAM") as dr2,
            tc.tile_pool(name="dram3", bufs=1, space="DRAM") as dr3,
            tc.tile_pool(name="dram4", bufs=1, space="DRAM") as dr4,
        ):
            # ---- constants to SBUF ----
            w1_sb = cp.tile([P, 4 * D], bf16)
            for k in range(4):
                nc.sync.dma_start(w1_sb[:, k * D:(k + 1) * D],
                                  w1_d[k * P:(k + 1) * P, :])
            w2_sb = cp.tile([P, D], bf16)
            nc.sync.dma_start(w2_sb[:], w2_d[:])
            wl_sb = cp.tile([P, NCLS], bf16)
            nc.sync.dma_start(wl_sb[:], wl_d[:])
            b1_sb = cp.tile([P, 1], f32)
            nc.sync.dma_start(b1_sb[:], b1_d[:])
            b2_sb = cp.tile([P, 1], f32)
            nc.sync.dma_start(b2_sb[:], b2_d[:])
            bl_sb = cp.tile([P, NCLS], f32)
            nc.sync.dma_start(bl_sb[:], bl_d[:])
            gidx_sb = cp.tile([P, PAIRS * (LO_COLS + HI_COLS)], mybir.dt.int16)
            nc.sync.dma_start(gidx_sb[:], gidx_d[:])
            cnt_sb = cp.tile([P, 2 * PAIRS], mybir.dt.int32)
            nc.sync.dma_start(cnt_sb[:], cnt_d[:])
            # per-layer local XW tables kept in SBUF for the self-loop matmul
            t_all = [cp.tile([P, NLOC], bf16, name="t_all0"),
                     cp.tile([P, NLOC], bf16, name="t_all1")]

            t_loc = [dr1.tile([NLOC, D], bf16, name="t_loc0"),
                     dr2.tile([NLOC, D], bf16, name="t_loc1")]
            t_fullA = [dr3.tile([LO_LIM, D], bf16, name="t_fullA0", tag="a0",
                                addr_space="Shared"),
                       dr3.tile([LO_LIM, D], bf16, name="t_fullA1", tag="a1",
                                addr_space="Shared")]
            t_fullB = [dr4.tile([VTOT - LO_LIM, D], bf16, name="t_fullB0",
                                tag="b0", addr_space="Shared"),
                       dr4.tile([VTOT - LO_LIM, D], bf16, name="t_fullB1",
                                tag="b1", addr_space="Shared")]

            # warm-up collective: absorbs the NRT first-collective barrier
            # while XW1 runs (dedicated scratch, no deps on real work)
            warm_in = dr1.tile([1, D], bf16, name="warm_in", tag="wi")
            warm_out = dr1.tile([NCORES, D], bf16, name="warm_out", tag="wo")
            nc.gpsimd.collective_compute(
                "AllGather",
                mybir.AluOpType.bypass,
                replica_groups=[list(range(NCORES))],
                ins=[warm_in[:, :].opt()],
                outs=[warm_out[:, :].opt()],
            )

            # zero all gather buffers once so count-trimmed (ungathered)
            # tail positions hold finite values (NaN*0 = NaN in the PE)
            for i in range(DEPTH + 1):
                gz = gp.tile([P, 2 * CLO * D], bf16, tag="glo", bufs=DEPTH + 1,
                             name=f"gz_lo{i}")
                nc.vector.memset(gz[:], 0)
            for i in range(HDEPTH + 2):
                gz = gp.tile([P, 2 * CHI * D], bf16, tag="ghi", bufs=HDEPTH + 2,
                             name=f"gz_hi{i}")
                nc.vector.memset(gz[:], 0)

            # ---- phase 0: XW1 ----
            xt_sb = xtp.tile([P, 4 * NLOC], bf16)
            for k in range(4):
                nc.sync.dma_start(xt_sb[:, k * NLOC:k * NLOC + ROWS_A],
                                  xt_d[k * P:(k + 1) * P, 0:ROWS_A])
            for k in range(4):
                nc.sync.dma_start(xt_sb[:, k * NLOC + ROWS_A:(k + 1) * NLOC],
                                  xt_d[k * P:(k + 1) * P, ROWS_A:NLOC])
            for t in range(TILES):
                ps = xwps.tile([P, D], f32, tag="xw")
                for k in range(4):
                    nc.tensor.matmul(
                        out=ps[:],
                        lhsT=xt_sb[:, k * NLOC + t * P: k * NLOC + (t + 1) * P],
                        rhs=w1_sb[:, k * D:(k + 1) * D],
                        start=(k == 0), stop=(k == 3),
                    )
                nc.scalar.activation(out=t_all[0][:, t * P:(t + 1) * P],
                                     in_=ps[:],
                                     func=mybir.ActivationFunctionType.Copy)
                nc.sync.dma_start(t_loc[0][t * P:(t + 1) * P, :],
                                  t_all[0][:, t * P:(t + 1) * P])

            def allgather(li):
                nc.gpsimd.collective_compute(
                    "AllGather",
                    mybir.AluOpType.bypass,
                    replica_groups=[list(range(NCORES))],
                    ins=[t_loc[li][0:ROWS_A, :].opt()],
                    outs=[t_fullA[li][:, :].opt()],
                )
                nc.gpsimd.collective_compute(
                    "AllGather",
                    mybir.AluOpType.bypass,
                    replica_groups=[list(range(NCORES))],
                    ins=[t_loc[li][ROWS_A:NLOC, :].opt()],
                    outs=[t_fullB[li][:, :].opt()],
                )

            def gather_half(li, pp, nt, half):
                tag = "glo" if half == 0 else "ghi"
                nbuf = DEPTH + 1 if half == 0 else HDEPTH + 2
                cap = LO_CAP if half == 0 else HI_CAP
                nch = CLO if half == 0 else CHI
                g = gp.tile([P, 2 * nch * D], bf16, tag=tag, bufs=nbuf,
                            name=f"g{tag}{li}_{pp}")
                ni = nt * cap
                src = (t_fullA[li][:, :] if half == 0
                       else t_fullB[li][:, :])
                c0 = pp * (LO_COLS + HI_COLS) + (0 if half == 0 else LO_COLS)
                qn = qctr[0] % 4
                ni_reg = cnt_regs[qn]
                nc.gpsimd.reg_load(ni_reg,
                                   cnt_sb[0:1, 2 * pp + half:2 * pp + half + 1])
                gi = nc.gpsimd.dma_gather(
                    g[:, :nt * nch * D].rearrange("p (c d) -> p c d", d=D),
                    src,
                    gidx_sb[:, c0:c0 + ni // 16],
                    ni, ni_reg, D, single_packet=False,
                    queue_num=qn,
                )
                qctr[0] += 1
                if first_gather[0]:
                    add_dep_helper(gi.ins, lib.ins, reason="lib before gather")
                    first_gather[0] = False
                return g

            def agg_tile(li, qq, ti, g_lo, g_hi):
                t = 2 * qq + ti
                s_sb = sp.tile([P, (CPT + 1) * P], bf16, tag="s", bufs=5)
                nc.sync.dma_start(s_sb[:], sval_d[t, :, :])
                agg = aggps.tile([P, P], f32, tag="agg")
                for c in range(CLO):
                    nc.tensor.matmul(
                        out=agg[:],
                        lhsT=g_lo[:, (ti * CLO + c) * D:(ti * CLO + c + 1) * D],
                        rhs=s_sb[:, c * P:(c + 1) * P],
                        start=(c == 0), stop=False,
                    )
                for c in range(CHI):
                    nc.tensor.matmul(
                        out=agg[:],
                        lhsT=g_hi[:, (ti * CHI + c) * D:(ti * CHI + c + 1) * D],
                        rhs=s_sb[:, (CLO + c) * P:(CLO + c + 1) * P],
                        start=False, stop=False,
                    )
                nc.tensor.matmul(
                    out=agg[:],
                    lhsT=t_all[li][:, t * P:(t + 1) * P],
                    rhs=s_sb[:, CPT * P:(CPT + 1) * P],
                    start=False, stop=True,
                )
                return agg

            # ---- phase 1+2: layer-1 aggregation + XW2 ----
            allgather(0)
            glo_buf = {}
            ghi_buf = {}
            for pp in range(PAIRS + DEPTH):
                nt_pp = 2 if pp < PAIRS and 2 * pp + 1 < TILES else 1
                if pp < PAIRS:
                    glo_buf[pp] = (gather_half(0, pp, nt_pp, 0), nt_pp)
                hh = pp - DEPTH + HDEPTH
                if 0 <= hh < PAIRS:
                    nt_hh = 2 if 2 * hh + 1 < TILES else 1
                    ghi_buf[hh] = gather_half(0, hh, nt_hh, 1)
                qq = pp - DEPTH
                if qq < 0:
                    continue
                g_lo, nt = glo_buf.pop(qq)
                g_hi = ghi_buf.pop(qq)
                for ti in range(nt):
                    t = 2 * qq + ti
                    agg = agg_tile(0, qq, ti, g_lo, g_hi)
                    h1t = hp.tile([P, P], bf16, tag="h")
                    nc.scalar.activation(out=h1t[:], in_=agg[:],
                                         func=mybir.ActivationFunctionType.Relu,
                                         bias=b1_sb[:])
                    ps2 = xwps.tile([P, D], f32, tag="xw2")
                    nc.tensor.matmul(out=ps2[:], lhsT=h1t[:], rhs=w2_sb[:],
                                     start=True, stop=True)
                    nc.vector.tensor_copy(out=t_all[1][:, t * P:(t + 1) * P],
                                          in_=ps2[:])
                    nc.sync.dma_start(t_loc[1][t * P:(t + 1) * P, :],
                                      t_all[1][:, t * P:(t + 1) * P])

            # ---- phase 3+4: layer-2 aggregation + head ----
            allgather(1)
            glo_buf = {}
            ghi_buf = {}
            for pp in range(PAIRS + DEPTH):
                nt_pp = 2 if pp < PAIRS and 2 * pp + 1 < TILES else 1
                if pp < PAIRS:
                    glo_buf[pp] = (gather_half(1, pp, nt_pp, 0), nt_pp)
                hh = pp - DEPTH + HDEPTH
                if 0 <= hh < PAIRS:
                    nt_hh = 2 if 2 * hh + 1 < TILES else 1
                    ghi_buf[hh] = gather_half(1, hh, nt_hh, 1)
                qq = pp - DEPTH
                if qq < 0:
                    continue
                g_lo, nt = glo_buf.pop(qq)
                g_hi = ghi_buf.pop(qq)
                for ti in range(nt):
                    t = 2 * qq + ti
                    agg = agg_tile(1, qq, ti, g_lo, g_hi)
                    h2t = hp.tile([P, P], bf16, tag="h")
                    nc.scalar.activation(out=h2t[:], in_=agg[:],
                                         func=mybir.ActivationFunctionType.Relu,
                                         bias=b2_sb[:])
                    lg = lgps.tile([P, NCLS], f32, tag="lg")
                    nc.tensor.matmul(out=lg[:], lhsT=h2t[:], rhs=wl_sb[:],
                                     start=True, stop=True)
                    l_sb = hdp.tile([P, NCLS], f32, tag="l")
                    nc.vector.tensor_add(out=l_sb[:], in0=lg[:], in1=bl_sb[:])
                    nmx = hdp.tile([P, 1], f32, tag="nmx")
                    nc.vector.reduce_max(out=nmx[:], in_=l_sb[:],
                                         axis=mybir.AxisListType.X, negate=True)
                    e_sb = hdp.tile([P, NCLS], f32, tag="e")
                    nc.scalar.activation(out=e_sb[:], in_=l_sb[:],
                                         func=mybir.ActivationFunctionType.Exp,
                                         bias=nmx[:])
                    sm = hdp.tile([P, 1], f32, tag="sm")
                    nc.vector.reduce_sum(out=sm[:], in_=e_sb[:],
                                         axis=mybir.AxisListType.X)
                    rs = hdp.tile([P, 1], f32, tag="rs")
                    nc.vector.reciprocal(out=rs[:], in_=sm[:])
                    pr = hdp.tile([P, NCLS], f32, tag="pr")
                    nc.scalar.activation(out=pr[:], in_=e_sb[:],
                                         func=mybir.ActivationFunctionType.Copy,
                                         scale=rs[:])
                    nc.sync.dma_start(out_d[t * P:(t + 1) * P, :], pr[:])

    nc.compile()
    return nc


def _preprocess(x, edge_index, W1, b1, W2, b2, Wlin, blin):
    """Host-side graph preprocessing -> per-core input dicts + slot maps."""
    x = np.asarray(x, np.float32)
    ei = np.asarray(edge_index)
    row = ei[0].astype(np.int64)
    col = ei[1].astype(np.int64)

    deg = np.bincount(col, minlength=N).astype(np.float32) + 2.0
    dis = 1.0 / np.sqrt(deg)
    norm_e = (dis[row] * dis[col]).astype(np.float32)
    selfval = (2.0 * dis * dis).astype(np.float32)

    indeg = np.bincount(col, minlength=N)  # per-node in-edges (no self)

    # balanced node->bin assignment (bins = core*TILES + tile), snake by degree
    NB = NCORES * TILES
    order = np.argsort(-indeg, kind="stable")
    bin_of_node = np.empty(N, np.int64)
    full_rounds = N // NB
    rem = N - full_rounds * NB
    fwd = np.arange(NB)
    bwd = fwd[::-1]
    seq = []
    for r in range(full_rounds):
        seq.append(fwd if r % 2 == 0 else bwd)
    if rem:
        seq.append((fwd if full_rounds % 2 == 0 else bwd)[:rem])
    seq = np.concatenate(seq)
    bin_of_node[order] = seq

    # within each pair of tiles, put the tile with more in-edges first
    bin_edges = np.bincount(bin_of_node[col], minlength=NB)
    perm = np.arange(NB)
    for c in range(NCORES):
        for pp in range(TILES // 2):
            b0 = c * TILES + 2 * pp
            if bin_edges[b0] < bin_edges[b0 + 1]:
                perm[b0], perm[b0 + 1] = b0 + 1, b0
    inv = np.empty(NB, np.int64)
    inv[perm] = np.arange(NB)
    bin_of_node = inv[bin_of_node]

    pos_in_bin = np.empty(N, np.int64)
    srt = np.argsort(bin_of_node, kind="stable")
    cnt = np.bincount(bin_of_node, minlength=NB)
    assert cnt.max() <= P
    starts = np.zeros(NB + 1, np.int64)
    np.cumsum(cnt, out=starts[1:])
    pos_in_bin[srt] = np.arange(N) - starts[bin_of_node[srt]]

    bin_edge_cnt = np.bincount(bin_of_node[col], minlength=NB)
    assert bin_edge_cnt.max() <= ECAP, (
        f"bin edge overflow: {bin_edge_cnt.max()} > {ECAP}")

    core_of_node = bin_of_node // TILES
    tile_of_node = bin_of_node % TILES
    # table row: AG-A region holds tiles 0..TILES_A-1 of every core, then AG-B
    gslot = np.where(
        tile_of_node < TILES_A,
        core_of_node * ROWS_A + tile_of_node * P + pos_in_bin,
        LO_LIM + core_of_node * ROWS_B + (tile_of_node - TILES_A) * P + pos_in_bin,
    )

    # per-edge: destination bin + dest position; source table slot
    e_bin = bin_of_node[col]
    e_dpos = pos_in_bin[col]
    e_src = gslot[row]

    # group edges by bin
    e_order = np.argsort(e_bin, kind="stable")
    eb = e_bin[e_order]
    ed = e_dpos[e_order]
    es = e_src[e_order]
    en = norm_e[e_order]
    bstarts = np.searchsorted(eb, np.arange(NB + 1))

    in_maps = []
    for c in range(NCORES):
        gidx = np.zeros((P, PAIRS * (LO_COLS + HI_COLS)), np.int16)
        sval_f32 = np.zeros((TILES, P, (CPT + 1) * P), np.float32)
        cntv = np.ones((2 * PAIRS,), np.int32)
        for pp in range(PAIRS):
            nt = 2 if 2 * pp + 1 < TILES else 1
            # per tile: (rel_idx, dpos, norm) for lo and hi halves
            halves = {0: [], 1: []}
            for ti in range(nt):
                t = 2 * pp + ti
                b = c * TILES + t
                lo_f, hi_f = bstarts[b], bstarts[b + 1]
                srcs = es[lo_f:hi_f]
                dposs = ed[lo_f:hi_f]
                nrm = en[lo_f:hi_f]
                ne = len(srcs)
                is_lo = srcs < LO_LIM
                lo_n = int(is_lo.sum())
                assert lo_n <= LO_CAP and ne - lo_n <= HI_CAP, (c, t, ne, lo_n)
                for half in (0, 1):
                    sel = is_lo if half == 0 else ~is_lo
                    hs, hd, hn = srcs[sel], dposs[sel], nrm[sel]
                    rel = hs if half == 0 else hs - LO_LIM
                    o3 = np.argsort(rel, kind="stable")
                    rel, hd, hn = rel[o3], hd[o3], hn[o3]
                    # dedup repeated sources: gather once, S row gets all the
                    # (dest, norm) entries of the duplicates
                    uniq, uinv = np.unique(rel, return_inverse=True)
                    halves[half].append((uniq, uinv, hd, hn))
            for half in (0, 1):
                cap = LO_CAP if half == 0 else HI_CAP
                flat = np.zeros(nt * cap, np.int64)
                last_k = 0
                for ti in range(nt):
                    t = 2 * pp + ti
                    uniq, uinv, hd, hn = halves[half][ti]
                    k = len(uniq)
                    last_k = k
                    flat[ti * cap: ti * cap + k] = uniq
                    if k < cap:
                        # interior padding: repeat last valid idx (or 0)
                        flat[ti * cap + k: (ti + 1) * cap] = uniq[-1] if k else 0
                    # dense S values: tile-local position -> (chunk, partition)
                    cbase = 0 if half == 0 else CLO
                    cidx = cbase + uinv // P
                    pidx = uinv % P
                    np.add.at(sval_f32, (np.full(len(hd), t), pidx,
                                         cidx * P + hd), hn)
                cnt_ph = max(1, (nt - 1) * cap + last_k)
                cntv[2 * pp + half] = cnt_ph
                # trailing padding past the real count is -1: the ucode trims
                # it and the decode reserves ring space from the count reg,
                # so both sides agree on the descriptor count
                flat[cnt_ph:] = -1
                w = flat.reshape(len(flat) // 16, 16).T.astype(np.int16)
                c0 = pp * (LO_COLS + HI_COLS) + (0 if half == 0 else LO_COLS)
                gidx[:, c0:c0 + len(flat) // 16] = np.tile(w, (8, 1))
        # self-loop diagonal chunk (chunk CPT of each tile)
        mine = np.where(core_of_node == c)[0]
        lslot = tile_of_node[mine] * P + pos_in_bin[mine]
        for t in range(TILES):
            sel = tile_of_node[mine] == t
            nodes_t = mine[sel]
            pos_t = pos_in_bin[nodes_t]
            sval_f32[t, pos_t, CPT * P + pos_t] = selfval[nodes_t]
        sval = sval_f32.astype(ml_dtypes.bfloat16)
        # x slice, transposed, padded
        xt = np.zeros((FIN, NLOC), ml_dtypes.bfloat16)
        xt[:, lslot] = x[mine].T.astype(ml_dtypes.bfloat16)
        in_maps.append({
            "xt": xt,
            "w1": np.asarray(W1).astype(ml_dtypes.bfloat16),
            "w2": np.asarray(W2).astype(ml_dtypes.bfloat16),
            "wl": np.asarray(Wlin).astype(ml_dtypes.bfloat16),
            "b1": np.asarray(b1, np.float32).reshape(P, 1),
            "b2": np.asarray(b2, np.float32).reshape(P, 1),
            "bl": np.tile(np.asarray(blin, np.float32).reshape(1, NCLS), (P, 1)),
            "gidx": gidx,
            "sval": sval,
            "cnt": np.tile(cntv[None, :], (P, 1)),
        })
    return in_maps, core_of_node, tile_of_node, pos_in_bin


def kernel(x, edge_index, W1, b1, W2, b2, Wlin, blin):
    global _PROGRAM, LAST_EXEC_NS, LAST_RESULT
    in_maps, core_of, tile_of, pos_of = _preprocess(
        x, edge_index, W1, b1, W2, b2, Wlin, blin)
    if _PROGRAM is None:
        _PROGRAM = _build_program()
    res = run_bass_kernel_spmd(
        _PROGRAM, in_maps, core_ids=list(range(NCORES)), trace=TRACE)
    LAST_EXEC_NS = res.exec_time_ns
    LAST_RESULT = res
    out = np.empty((N, NCLS), np.float32)
    per_core = [res.results[c]["probs"] for c in range(NCORES)]
    lslot = tile_of * P + pos_of
    for c in range(NCORES):
        mine = np.where(core_of == c)[0]
        out[mine] = per_core[c][lslot[mine]]
    return out


# revision 15
# speedup vs baseline: 1.6840x; 1.0124x over previous
"""2-layer GCN (improved=True) + linear head + softmax on 8 Trainium2 cores.

Strategy (dest-node partitioning):
- Nodes assigned to 8 cores x 49 tiles x 128 slots via balanced bin-packing on
  in-degree (max 2176 in-edges per tile; self-loops are NOT gathered).
- Per layer: each core computes XW for its slots (node-major [n,128]),
  AllGather replicates the table to every core's HBM (Shared scratch for the
  fast collective path), then per dest-tile-pair the core gathers source rows
  with dma_gather (lo window 2048 idx / hi window 2304 idx, trailing -1
  padding is trimmed by the ucode) and scatter-adds them with one-hot matmuls
  agg^T[d, n] += G_chunk^T[d, e] @ S_chunk[e, n].
- S chunks are built ON-CHIP by the vector engine from compact per-chunk
  (dest-pos, norm) columns: S[e, n] = (iota[n] == dpos[e]) * val[e]; no dense
  S matrices ever touch HBM.
- The self-loop term (norm 2*dis^2) is one extra matmul per tile with
  lhsT = the tile's own XW output (kept in SBUF) and a diagonal S built the
  same way (dpos = arange, val = 2*dis^2).
- Gather calls rotate across all 4 SWDGE queues so descriptor generation for
  different calls can overlap on different GPSIMD core pairs.
- Head: logits = H2 @ Wlin + blin, softmax over 8 classes, on-chip.

kernel() is self-contained: host-side numpy does all graph preprocessing;
the device program is identical on all 8 cores, only data differs.
"""
import sys

sys.path.insert(0, "/opt/trn_rl_repo")

import numpy as np
import ml_dtypes

import concourse.bass as bass
import concourse.bacc as bacc
import concourse.mybir as mybir
import concourse.tile as tile
from concourse.tile_rust import add_dep_helper
from concourse.bass_utils import run_bass_kernel_spmd
from concourse.library_config import mlp

# problem constants
N = 50000
E = 800000
FIN = 512
D = 128
NCLS = 8
NCORES = 8

# sharding constants
P = 128
TILES = 49
NLOC = TILES * P            # 6272 slots per core
VTOT = NCORES * NLOC        # 50176 table rows
LO_CAP = 1152               # per-tile lo-window edge cap (9 chunks)
HI_CAP = 1152               # per-tile hi-window edge cap (9 chunks)
ECAP = LO_CAP + HI_CAP      # 2304 in-edges per tile
CLO = LO_CAP // P           # 9 lo chunks per tile
CHI = HI_CAP // P           # 9 hi chunks per tile
CPT = CLO + CHI             # 18 edge chunks per tile
TILES_A = 25                # tiles in AG phase A
ROWS_A = TILES_A * P        # 3328 rows/core in phase A
LO_LIM = NCORES * ROWS_A    # 26624: lo gathers read only the AG-A region
TILES_B = TILES - TILES_A
ROWS_B = TILES_B * P
PAIRS = (TILES + 1) // 2
DEPTH = 7                   # lo-gather software-pipeline depth (pairs)
HDEPTH = 2                  # hi-gather prefetch depth (pairs)
LO_COLS = 2 * LO_CAP // 16  # 128 int16 idx cols per pair (lo)
HI_COLS = 2 * HI_CAP // 16  # 144 int16 idx cols per pair (hi)
SCOLS = PAIRS * (2 * CPT) + TILES  # dpos/val columns (incl self diag)

TRACE = False
LAST_EXEC_NS = None
LAST_RESULT = None

_PROGRAM = None


def _build_program():
    nc = bacc.Bacc(None, target_bir_lowering=False, num_swdge_queues=4,
                   dynamic_dma_scratch_size=32768)
    f32 = mybir.dt.float32
    bf16 = mybir.dt.bfloat16

    xt_d = nc.dram_tensor("xt", [FIN, NLOC], bf16, kind="ExternalInput")
    w1_d = nc.dram_tensor("w1", [FIN, D], bf16, kind="ExternalInput")
    w2_d = nc.dram_tensor("w2", [D, D], bf16, kind="ExternalInput")
    wl_d = nc.dram_tensor("wl", [D, NCLS], bf16, kind="ExternalInput")
    b1_d = nc.dram_tensor("b1", [P, 1], f32, kind="ExternalInput")
    b2_d = nc.dram_tensor("b2", [P, 1], f32, kind="ExternalInput")
    bl_d = nc.dram_tensor("bl", [P, NCLS], f32, kind="ExternalInput")
    gidx_d = nc.dram_tensor("gidx", [P, PAIRS * (LO_COLS + HI_COLS)],
                            mybir.dt.int16, kind="ExternalInput")
    sval_d = nc.dram_tensor("sval", [TILES, P, (CPT + 1) * P], bf16,
                            kind="ExternalInput")
    cnt_d = nc.dram_tensor("cnt", [P, 2 * PAIRS], mybir.dt.int32,
                           kind="ExternalInput")
    out_d = nc.dram_tensor("probs", [NLOC, NCLS], f32, kind="ExternalOutput")

    with tile.TileContext(nc) as tc:
        lib = nc.gpsimd.load_library(mlp)
        first_gather = [True]
        qctr = [0]
        cnt_regs = [nc.gpsimd.alloc_register(f"cntreg{i}") for i in range(4)]

        with (
            tc.tile_pool(name="const", bufs=1) as cp,
            tc.tile_pool(name="xtp", bufs=1) as xtp,
            tc.tile_pool(name="gpool", bufs=4) as gp,
            tc.tile_pool(name="spool", bufs=8) as sp,
            tc.tile_pool(name="hpool", bufs=3) as hp,
            tc.tile_pool(name="headp", bufs=3) as hdp,
            tc.tile_pool(name="xwps", bufs=2, space="PSUM") as xwps,
            tc.tile_pool(name="aggps", bufs=2, space="PSUM") as aggps,
            tc.tile_pool(name="lgps", bufs=2, space="PSUM") as lgps,
            tc.tile_pool(name="dram1", bufs=1, space="DRAM") as dr1,
            tc.tile_pool(name="dram2", bufs=1, space="DRAM") as dr2,
            tc.tile_pool(name="dram3", bufs=1, space="DRAM") as dr3,
            tc.tile_pool(name="dram4", bufs=1, space="DRAM") as dr4,
        ):
            # ---- constants to SBUF ----
            w1_sb = cp.tile([P, 4 * D], bf16)
            for k in range(4):
                nc.sync.dma_start(w1_sb[:, k * D:(k + 1) * D],
                                  w1_d[k * P:(k + 1) * P, :])
            w2_sb = cp.tile([P, D], bf16)
            nc.sync.dma_start(w2_sb[:], w2_d[:])
            wl_sb = cp.tile([P, NCLS], bf16)
            nc.sync.dma_start(wl_sb[:], wl_d[:])
            b1_sb = cp.tile([P, 1], f32)
            nc.sync.dma_start(b1_sb[:], b1_d[:])
            b2_sb = cp.tile([P, 1], f32)
            nc.sync.dma_start(b2_sb[:], b2_d[:])
            bl_sb = cp.tile([P, NCLS], f32)
            nc.sync.dma_start(bl_sb[:], bl_d[:])
            gidx_sb = cp.tile([P, PAIRS * (LO_COLS + HI_COLS)], mybir.dt.int16)
            nc.sync.dma_start(gidx_sb[:], gidx_d[:])
            cnt_sb = cp.tile([P, 2 * PAIRS], mybir.dt.int32)
            nc.sync.dma_start(cnt_sb[:], cnt_d[:])
            # per-layer local XW tables kept in SBUF for the self-loop matmul
            t_all = [cp.tile([P, NLOC], bf16, name="t_all0"),
                     cp.tile([P, NLOC], bf16, name="t_all1")]

            t_loc = [dr1.tile([NLOC, D], bf16, name="t_loc0"),
                     dr2.tile([NLOC, D], bf16, name="t_loc1")]
            t_fullA = [dr3.tile([LO_LIM, D], bf16, name="t_fullA0", tag="a0",
                                addr_space="Shared"),
                       dr3.tile([LO_LIM, D], bf16, name="t_fullA1", tag="a1",
                                addr_space="Shared")]
            t_fullB = [dr4.tile([VTOT - LO_LIM, D], bf16, name="t_fullB0",
                                tag="b0", addr_space="Shared"),
                       dr4.tile([VTOT - LO_LIM, D], bf16, name="t_fullB1",
                                tag="b1", addr_space="Shared")]

            # zero all gather buffers once so count-trimmed (ungathered)
            # tail positions hold finite values (NaN*0 = NaN in the PE)
            for i in range(DEPTH + 1):
                gz = gp.tile([P, 2 * CLO * D], bf16, tag="glo", bufs=DEPTH + 1,
                             name=f"gz_lo{i}")
                nc.vector.memset(gz[:], 0)
            for i in range(HDEPTH + 2):
                gz = gp.tile([P, 2 * CHI * D], bf16, tag="ghi", bufs=HDEPTH + 2,
                             name=f"gz_hi{i}")
                nc.vector.memset(gz[:], 0)

            # ---- phase 0: XW1 ----
            xt_sb = xtp.tile([P, 4 * NLOC], bf16)
            for k in range(4):
                nc.sync.dma_start(xt_sb[:, k * NLOC:k * NLOC + ROWS_A],
                                  xt_d[k * P:(k + 1) * P, 0:ROWS_A])
            for k in range(4):
                nc.sync.dma_start(xt_sb[:, k * NLOC + ROWS_A:(k + 1) * NLOC],
                                  xt_d[k * P:(k + 1) * P, ROWS_A:NLOC])
            for t in range(TILES):
                ps = xwps.tile([P, D], f32, tag="xw")
                for k in range(4):
                    nc.tensor.matmul(
                        out=ps[:],
                        lhsT=xt_sb[:, k * NLOC + t * P: k * NLOC + (t + 1) * P],
                        rhs=w1_sb[:, k * D:(k + 1) * D],
                        start=(k == 0), stop=(k == 3),
                    )
                nc.scalar.activation(out=t_all[0][:, t * P:(t + 1) * P],
                                     in_=ps[:],
                                     func=mybir.ActivationFunctionType.Copy)
                nc.sync.dma_start(t_loc[0][t * P:(t + 1) * P, :],
                                  t_all[0][:, t * P:(t + 1) * P])

            def allgather(li):
                nc.gpsimd.collective_compute(
                    "AllGather",
                    mybir.AluOpType.bypass,
                    replica_groups=[list(range(NCORES))],
                    ins=[t_loc[li][0:ROWS_A, :].opt()],
                    outs=[t_fullA[li][:, :].opt()],
                )
                nc.gpsimd.collective_compute(
                    "AllGather",
                    mybir.AluOpType.bypass,
                    replica_groups=[list(range(NCORES))],
                    ins=[t_loc[li][ROWS_A:NLOC, :].opt()],
                    outs=[t_fullB[li][:, :].opt()],
                )

            def gather_half(li, pp, nt, half):
                tag = "glo" if half == 0 else "ghi"
                nbuf = DEPTH + 1 if half == 0 else HDEPTH + 2
                cap = LO_CAP if half == 0 else HI_CAP
                nch = CLO if half == 0 else CHI
                g = gp.tile([P, 2 * nch * D], bf16, tag=tag, bufs=nbuf,
                            name=f"g{tag}{li}_{pp}")
                ni = nt * cap
                src = (t_fullA[li][:, :] if half == 0
                       else t_fullB[li][:, :])
                c0 = pp * (LO_COLS + HI_COLS) + (0 if half == 0 else LO_COLS)
                qn = qctr[0] % 4
                ni_reg = cnt_regs[qn]
                nc.gpsimd.reg_load(ni_reg,
                                   cnt_sb[0:1, 2 * pp + half:2 * pp + half + 1])
                gi = nc.gpsimd.dma_gather(
                    g[:, :nt * nch * D].rearrange("p (c d) -> p c d", d=D),
                    src,
                    gidx_sb[:, c0:c0 + ni // 16],
                    ni, ni_reg, D, single_packet=False,
                    queue_num=qn,
                )
                qctr[0] += 1
                if first_gather[0]:
                    add_dep_helper(gi.ins, lib.ins, reason="lib before gather")
                    first_gather[0] = False
                return g

            def agg_tile(li, qq, ti, g_lo, g_hi):
                t = 2 * qq + ti
                s_sb = sp.tile([P, (CPT + 1) * P], bf16, tag="s", bufs=4)
                nc.sync.dma_start(s_sb[:], sval_d[t, :, :])
                agg = aggps.tile([P, P], f32, tag="agg")
                for c in range(CLO):
                    nc.tensor.matmul(
                        out=agg[:],
                        lhsT=g_lo[:, (ti * CLO + c) * D:(ti * CLO + c + 1) * D],
                        rhs=s_sb[:, c * P:(c + 1) * P],
                        start=(c == 0), stop=False,
                    )
                for c in range(CHI):
                    nc.tensor.matmul(
                        out=agg[:],
                        lhsT=g_hi[:, (ti * CHI + c) * D:(ti * CHI + c + 1) * D],
                        rhs=s_sb[:, (CLO + c) * P:(CLO + c + 1) * P],
                        start=False, stop=False,
                    )
                nc.tensor.matmul(
                    out=agg[:],
                    lhsT=t_all[li][:, t * P:(t + 1) * P],
                    rhs=s_sb[:, CPT * P:(CPT + 1) * P],
                    start=False, stop=True,
                )
                return agg

            # ---- phase 1+2: layer-1 aggregation + XW2 ----
            allgather(0)
            glo_buf = {}
            ghi_buf = {}
            for pp in range(PAIRS + DEPTH):
                nt_pp = 2 if pp < PAIRS and 2 * pp + 1 < TILES else 1
                if pp < PAIRS:
                    glo_buf[pp] = (gather_half(0, pp, nt_pp, 0), nt_pp)
                hh = pp - DEPTH + HDEPTH
                if 0 <= hh < PAIRS:
                    nt_hh = 2 if 2 * hh + 1 < TILES else 1
                    ghi_buf[hh] = gather_half(0, hh, nt_hh, 1)
                qq = pp - DEPTH
                if qq < 0:
                    continue
                g_lo, nt = glo_buf.pop(qq)
                g_hi = ghi_buf.pop(qq)
                for ti in range(nt):
                    t = 2 * qq + ti
                    agg = agg_tile(0, qq, ti, g_lo, g_hi)
                    h1t = hp.tile([P, P], bf16, tag="h")
                    nc.scalar.activation(out=h1t[:], in_=agg[:],
                                         func=mybir.ActivationFunctionType.Relu,
                                         bias=b1_sb[:])
                    ps2 = xwps.tile([P, D], f32, tag="xw2")
                    nc.tensor.matmul(out=ps2[:], lhsT=h1t[:], rhs=w2_sb[:],
                                     start=True, stop=True)
                    nc.vector.tensor_copy(out=t_all[1][:, t * P:(t + 1) * P],
                                          in_=ps2[:])
                    nc.sync.dma_start(t_loc[1][t * P:(t + 1) * P, :],
                                      t_all[1][:, t * P:(t + 1) * P])

            # ---- phase 3+4: layer-2 aggregation + head ----
            allgather(1)
            glo_buf = {}
            ghi_buf = {}
            for pp in range(PAIRS + DEPTH):
                nt_pp = 2 if pp < PAIRS and 2 * pp + 1 < TILES else 1
                if pp < PAIRS:
                    glo_buf[pp] = (gather_half(1, pp, nt_pp, 0), nt_pp)
                hh = pp - DEPTH + HDEPTH
                if 0 <= hh < PAIRS:
                    nt_hh = 2 if 2 * hh + 1 < TILES else 1
                    ghi_buf[hh] = gather_half(1, hh, nt_hh, 1)
                qq = pp - DEPTH
                if qq < 0:
                    continue
                g_lo, nt = glo_buf.pop(qq)
                g_hi = ghi_buf.pop(qq)
                for ti in range(nt):
                    t = 2 * qq + ti
                    agg = agg_tile(1, qq, ti, g_lo, g_hi)
                    h2t = hp.tile([P, P], bf16, tag="h")
                    nc.scalar.activation(out=h2t[:], in_=agg[:],
                                         func=mybir.ActivationFunctionType.Relu,
                                         bias=b2_sb[:])
                    lg = lgps.tile([P, NCLS], f32, tag="lg")
                    nc.tensor.matmul(out=lg[:], lhsT=h2t[:], rhs=wl_sb[:],
                                     start=True, stop=True)
                    l_sb = hdp.tile([P, NCLS], f32, tag="l")
                    nc.vector.tensor_add(out=l_sb[:], in0=lg[:], in1=bl_sb[:])
                    nmx = hdp.tile([P, 1], f32, tag="nmx")
                    nc.vector.reduce_max(out=nmx[:], in_=l_sb[:],
                                         axis=mybir.AxisListType.X, negate=True)
                    e_sb = hdp.tile([P, NCLS], f32, tag="e")
                    nc.scalar.activation(out=e_sb[:], in_=l_sb[:],
                                         func=mybir.ActivationFunctionType.Exp,
                                         bias=nmx[:])
                    sm = hdp.tile([P, 1], f32, tag="sm")
                    nc.vector.reduce_sum(out=sm[:], in_=e_sb[:],
                                         axis=mybir.AxisListType.X)
                    rs = hdp.tile([P, 1], f32, tag="rs")
                    nc.vector.reciprocal(out=rs[:], in_=sm[:])
                    pr = hdp.tile([P, NCLS], f32, tag="pr")
                    nc.scalar.activation(out=pr[:], in_=e_sb[:],
                                         func=mybir.ActivationFunctionType.Copy,
                                         scale=rs[:])
                    nc.sync.dma_start(out_d[t * P:(t + 1) * P, :], pr[:])

    nc.compile()
    return nc


def _preprocess(x, edge_index, W1, b1, W2, b2, Wlin, blin):
    """Host-side graph preprocessing -> per-core input dicts + slot maps."""
    x = np.asarray(x, np.float32)
    ei = np.asarray(edge_index)
    row = ei[0].astype(np.int64)
    col = ei[1].astype(np.int64)

    deg = np.bincount(col, minlength=N).astype(np.float32) + 2.0
    dis = 1.0 / np.sqrt(deg)
    norm_e = (dis[row] * dis[col]).astype(np.float32)
    selfval = (2.0 * dis * dis).astype(np.float32)

    indeg = np.bincount(col, minlength=N)  # per-node in-edges (no self)

    # balanced node->bin assignment (bins = core*TILES + tile), snake by degree
    NB = NCORES * TILES
    order = np.argsort(-indeg, kind="stable")
    bin_of_node = np.empty(N, np.int64)
    full_rounds = N // NB
    rem = N - full_rounds * NB
    fwd = np.arange(NB)
    bwd = fwd[::-1]
    seq = []
    for r in range(full_rounds):
        seq.append(fwd if r % 2 == 0 else bwd)
    if rem:
        seq.append((fwd if full_rounds % 2 == 0 else bwd)[:rem])
    seq = np.concatenate(seq)
    bin_of_node[order] = seq

    # within each pair of tiles, put the tile with more in-edges first
    bin_edges = np.bincount(bin_of_node[col], minlength=NB)
    perm = np.arange(NB)
    for c in range(NCORES):
        for pp in range(TILES // 2):
            b0 = c * TILES + 2 * pp
            if bin_edges[b0] < bin_edges[b0 + 1]:
                perm[b0], perm[b0 + 1] = b0 + 1, b0
    inv = np.empty(NB, np.int64)
    inv[perm] = np.arange(NB)
    bin_of_node = inv[bin_of_node]

    pos_in_bin = np.empty(N, np.int64)
    srt = np.argsort(bin_of_node, kind="stable")
    cnt = np.bincount(bin_of_node, minlength=NB)
    assert cnt.max() <= P
    starts = np.zeros(NB + 1, np.int64)
    np.cumsum(cnt, out=starts[1:])
    pos_in_bin[srt] = np.arange(N) - starts[bin_of_node[srt]]

    bin_edge_cnt = np.bincount(bin_of_node[col], minlength=NB)
    assert bin_edge_cnt.max() <= ECAP, (
        f"bin edge overflow: {bin_edge_cnt.max()} > {ECAP}")

    core_of_node = bin_of_node // TILES
    tile_of_node = bin_of_node % TILES
    # table row: AG-A region holds tiles 0..TILES_A-1 of every core, then AG-B
    gslot = np.where(
        tile_of_node < TILES_A,
        core_of_node * ROWS_A + tile_of_node * P + pos_in_bin,
        LO_LIM + core_of_node * ROWS_B + (tile_of_node - TILES_A) * P + pos_in_bin,
    )

    # per-edge: destination bin + dest position; source table slot
    e_bin = bin_of_node[col]
    e_dpos = pos_in_bin[col]
    e_src = gslot[row]

    # group edges by bin
    e_order = np.argsort(e_bin, kind="stable")
    eb = e_bin[e_order]
    ed = e_dpos[e_order]
    es = e_src[e_order]
    en = norm_e[e_order]
    bstarts = np.searchsorted(eb, np.arange(NB + 1))

    in_maps = []
    for c in range(NCORES):
        gidx = np.zeros((P, PAIRS * (LO_COLS + HI_COLS)), np.int16)
        sval_f32 = np.zeros((TILES, P, (CPT + 1) * P), np.float32)
        cntv = np.ones((2 * PAIRS,), np.int32)
        for pp in range(PAIRS):
            nt = 2 if 2 * pp + 1 < TILES else 1
            # per tile: (rel_idx, dpos, norm) for lo and hi halves
            halves = {0: [], 1: []}
            for ti in range(nt):
                t = 2 * pp + ti
                b = c * TILES + t
                lo_f, hi_f = bstarts[b], bstarts[b + 1]
                srcs = es[lo_f:hi_f]
                dposs = ed[lo_f:hi_f]
                nrm = en[lo_f:hi_f]
                ne = len(srcs)
                is_lo = srcs < LO_LIM
                lo_n = int(is_lo.sum())
                assert lo_n <= LO_CAP and ne - lo_n <= HI_CAP, (c, t, ne, lo_n)
                for half in (0, 1):
                    sel = is_lo if half == 0 else ~is_lo
                    hs, hd, hn = srcs[sel], dposs[sel], nrm[sel]
                    rel = hs if half == 0 else hs - LO_LIM
                    o3 = np.argsort(rel, kind="stable")
                    rel, hd, hn = rel[o3], hd[o3], hn[o3]
                    # dedup repeated sources: gather once, S row gets all the
                    # (dest, norm) entries of the duplicates
                    uniq, uinv = np.unique(rel, return_inverse=True)
                    halves[half].append((uniq, uinv, hd, hn))
            for half in (0, 1):
                cap = LO_CAP if half == 0 else HI_CAP
                flat = np.zeros(nt * cap, np.int64)
                last_k = 0
                for ti in range(nt):
                    t = 2 * pp + ti
                    uniq, uinv, hd, hn = halves[half][ti]
                    k = len(uniq)
                    last_k = k
                    flat[ti * cap: ti * cap + k] = uniq
                    if k < cap:
                        # interior padding: repeat last valid idx (or 0)
                        flat[ti * cap + k: (ti + 1) * cap] = uniq[-1] if k else 0
                    # dense S values: tile-local position -> (chunk, partition)
                    cbase = 0 if half == 0 else CLO
                    cidx = cbase + uinv // P
                    pidx = uinv % P
                    np.add.at(sval_f32, (np.full(len(hd), t), pidx,
                                         cidx * P + hd), hn)
                cnt_ph = max(1, (nt - 1) * cap + last_k)
                cntv[2 * pp + half] = cnt_ph
                # trailing padding past the real count is -1: the ucode trims
                # it and the decode reserves ring space from the count reg,
                # so both sides agree on the descriptor count
                flat[cnt_ph:] = -1
                w = flat.reshape(len(flat) // 16, 16).T.astype(np.int16)
                c0 = pp * (LO_COLS + HI_COLS) + (0 if half == 0 else LO_COLS)
                gidx[:, c0:c0 + len(flat) // 16] = np.tile(w, (8, 1))
        # self-loop diagonal chunk (chunk CPT of each tile)
        mine = np.where(core_of_node == c)[0]
        lslot = tile_of_node[mine] * P + pos_in_bin[mine]
        for t in range(TILES):
            sel = tile_of_node[mine] == t
            nodes_t = mine[sel]
            pos_t = pos_in_bin[nodes_t]
            sval_f32[t, pos_t, CPT * P + pos_t] = selfval[nodes_t]
        sval = sval_f32.astype(ml_dtypes.bfloat16)
        # x slice, transposed, padded
        xt = np.zeros((FIN, NLOC), ml_dtypes.bfloat16)
        xt[:, lslot] = x[mine].T.astype(ml_dtypes.bfloat16)
        in_maps.append({
            "xt": xt,
            "w1": np.asarray(W1).astype(ml_dtypes.bfloat16),
            "w2": np.asarray(W2).astype(ml_dtypes.bfloat16),
            "wl": np.asarray(Wlin).astype(ml_dtypes.bfloat16),
            "b1": np.asarray(b1, np.float32).reshape(P, 1),
            "b2": np.asarray(b2, np.float32).reshape(P, 1),
            "bl": np.tile(np.asarray(blin, np.float32).reshape(1, NCLS), (P, 1)),
            "gidx": gidx,
            "sval": sval,
            "cnt": np.tile(cntv[None, :], (P, 1)),
        })
    return in_maps, core_of_node, tile_of_node, pos_in_bin


def kernel(x, edge_index, W1, b1, W2, b2, Wlin, blin):
    global _PROGRAM, LAST_EXEC_NS, LAST_RESULT
    in_maps, core_of, tile_of, pos_of = _preprocess(
        x, edge_index, W1, b1, W2, b2, Wlin, blin)
    if _PROGRAM is None:
        _PROGRAM = _build_program()
    res = run_bass_kernel_spmd(
        _PROGRAM, in_maps, core_ids=list(range(NCORES)), trace=TRACE)
    LAST_EXEC_NS = res.exec_time_ns
    LAST_RESULT = res
    out = np.empty((N, NCLS), np.float32)
    per_core = [res.results[c]["probs"] for c in range(NCORES)]
    lslot = tile_of * P + pos_of
    for c in range(NCORES):
        mine = np.where(core_of == c)[0]
        out[mine] = per_core[c][lslot[mine]]
    return out


# revision 16
# speedup vs baseline: 1.7092x; 1.0150x over previous
"""2-layer GCN (improved=True) + linear head + softmax on 8 Trainium2 cores.

Strategy (dest-node partitioning):
- Nodes assigned to 8 cores x 49 tiles x 128 slots via balanced bin-packing on
  in-degree (max 2176 in-edges per tile; self-loops are NOT gathered).
- Per layer: each core computes XW for its slots (node-major [n,128]),
  AllGather replicates the table to every core's HBM (Shared scratch for the
  fast collective path), then per dest-tile-pair the core gathers source rows
  with dma_gather (lo window 2048 idx / hi window 2304 idx, trailing -1
  padding is trimmed by the ucode) and scatter-adds them with one-hot matmuls
  agg^T[d, n] += G_chunk^T[d, e] @ S_chunk[e, n].
- S chunks are built ON-CHIP by the vector engine from compact per-chunk
  (dest-pos, norm) columns: S[e, n] = (iota[n] == dpos[e]) * val[e]; no dense
  S matrices ever touch HBM.
- The self-loop term (norm 2*dis^2) is one extra matmul per tile with
  lhsT = the tile's own XW output (kept in SBUF) and a diagonal S built the
  same way (dpos = arange, val = 2*dis^2).
- Gather calls rotate across all 4 SWDGE queues so descriptor generation for
  different calls can overlap on different GPSIMD core pairs.
- Head: logits = H2 @ Wlin + blin, softmax over 8 classes, on-chip.

kernel() is self-contained: host-side numpy does all graph preprocessing;
the device program is identical on all 8 cores, only data differs.
"""
import sys

sys.path.insert(0, "/opt/trn_rl_repo")

import numpy as np
import ml_dtypes

import concourse.bass as bass
import concourse.bacc as bacc
import concourse.mybir as mybir
import concourse.tile as tile
from concourse.tile_rust import add_dep_helper
from concourse.bass_utils import run_bass_kernel_spmd
from concourse.library_config import mlp

# problem constants
N = 50000
E = 800000
FIN = 512
D = 128
NCLS = 8
NCORES = 8

# sharding constants
P = 128
TILES = 49
NLOC = TILES * P            # 6272 slots per core
VTOT = NCORES * NLOC        # 50176 table rows
LO_CAP = 1152               # per-tile lo-window edge cap (9 chunks)
HI_CAP = 1152               # per-tile hi-window edge cap (9 chunks)
ECAP = LO_CAP + HI_CAP      # 2304 in-edges per tile
CLO = LO_CAP // P           # 9 lo chunks per tile
CHI = HI_CAP // P           # 9 hi chunks per tile
CPT = CLO + CHI             # 18 edge chunks per tile
TILES_A = 25                # tiles in AG phase A
ROWS_A = TILES_A * P        # 3328 rows/core in phase A
LO_LIM = NCORES * ROWS_A    # 26624: lo gathers read only the AG-A region
TILES_B = TILES - TILES_A
ROWS_B = TILES_B * P
PAIRS = (TILES + 1) // 2
DEPTH = 9                   # lo-gather software-pipeline depth (pairs)
HDEPTH = 3                  # hi-gather prefetch depth (pairs)
LO_COLS = 2 * LO_CAP // 16  # 128 int16 idx cols per pair (lo)
HI_COLS = 2 * HI_CAP // 16  # 144 int16 idx cols per pair (hi)
SCOLS = PAIRS * (2 * CPT) + TILES  # dpos/val columns (incl self diag)

TRACE = False
LAST_EXEC_NS = None
LAST_RESULT = None

_PROGRAM = None


def _build_program():
    nc = bacc.Bacc(None, target_bir_lowering=False, num_swdge_queues=4,
                   dynamic_dma_scratch_size=32768)
    f32 = mybir.dt.float32
    bf16 = mybir.dt.bfloat16

    xt_d = nc.dram_tensor("xt", [FIN, NLOC], bf16, kind="ExternalInput")
    w1_d = nc.dram_tensor("w1", [FIN, D], bf16, kind="ExternalInput")
    w2_d = nc.dram_tensor("w2", [D, D], bf16, kind="ExternalInput")
    wl_d = nc.dram_tensor("wl", [D, NCLS], bf16, kind="ExternalInput")
    b1_d = nc.dram_tensor("b1", [P, 1], f32, kind="ExternalInput")
    b2_d = nc.dram_tensor("b2", [P, 1], f32, kind="ExternalInput")
    bl_d = nc.dram_tensor("bl", [P, NCLS], f32, kind="ExternalInput")
    gidx_d = nc.dram_tensor("gidx", [P, PAIRS * (LO_COLS + HI_COLS)],
                            mybir.dt.int16, kind="ExternalInput")
    sval_d = nc.dram_tensor("sval", [TILES, P, (CPT + 1) * P], bf16,
                            kind="ExternalInput")
    cnt_d = nc.dram_tensor("cnt", [P, 2 * PAIRS], mybir.dt.int32,
                           kind="ExternalInput")
    out_d = nc.dram_tensor("probs", [NLOC, NCLS], f32, kind="ExternalOutput")

    with tile.TileContext(nc) as tc:
        lib = nc.gpsimd.load_library(mlp)
        first_gather = [True]
        qctr = [0]
        cnt_regs = [nc.gpsimd.alloc_register(f"cntreg{i}") for i in range(4)]

        with (
            tc.tile_pool(name="const", bufs=1) as cp,
            tc.tile_pool(name="xtp", bufs=1) as xtp,
            tc.tile_pool(name="gpool", bufs=4) as gp,
            tc.tile_pool(name="spool", bufs=8) as sp,
            tc.tile_pool(name="hpool", bufs=3) as hp,
            tc.tile_pool(name="headp", bufs=3) as hdp,
            tc.tile_pool(name="xwps", bufs=2, space="PSUM") as xwps,
            tc.tile_pool(name="aggps", bufs=2, space="PSUM") as aggps,
            tc.tile_pool(name="lgps", bufs=2, space="PSUM") as lgps,
            tc.tile_pool(name="dram1", bufs=1, space="DRAM") as dr1,
            tc.tile_pool(name="dram2", bufs=1, space="DRAM") as dr2,
            tc.tile_pool(name="dram3", bufs=1, space="DRAM") as dr3,
            tc.tile_pool(name="dram4", bufs=1, space="DRAM") as dr4,
        ):
            # ---- constants to SBUF ----
            w1_sb = cp.tile([P, 4 * D], bf16)
            for k in range(4):
                nc.sync.dma_start(w1_sb[:, k * D:(k + 1) * D],
                                  w1_d[k * P:(k + 1) * P, :])
            w2_sb = cp.tile([P, D], bf16)
            nc.sync.dma_start(w2_sb[:], w2_d[:])
            wl_sb = cp.tile([P, NCLS], bf16)
            nc.sync.dma_start(wl_sb[:], wl_d[:])
            b1_sb = cp.tile([P, 1], f32)
            nc.sync.dma_start(b1_sb[:], b1_d[:])
            b2_sb = cp.tile([P, 1], f32)
            nc.sync.dma_start(b2_sb[:], b2_d[:])
            bl_sb = cp.tile([P, NCLS], f32)
            nc.sync.dma_start(bl_sb[:], bl_d[:])
            gidx_sb = cp.tile([P, PAIRS * (LO_COLS + HI_COLS)], mybir.dt.int16)
            nc.sync.dma_start(gidx_sb[:], gidx_d[:])
            cnt_sb = cp.tile([P, 2 * PAIRS], mybir.dt.int32)
            nc.sync.dma_start(cnt_sb[:], cnt_d[:])
            # per-layer local XW tables kept in SBUF for the self-loop matmul
            t_all = [cp.tile([P, NLOC], bf16, name="t_all0"),
                     cp.tile([P, NLOC], bf16, name="t_all1")]

            t_loc = [dr1.tile([NLOC, D], bf16, name="t_loc0"),
                     dr2.tile([NLOC, D], bf16, name="t_loc1")]
            t_fullA = [dr3.tile([LO_LIM, D], bf16, name="t_fullA0", tag="a0",
                                addr_space="Shared"),
                       dr3.tile([LO_LIM, D], bf16, name="t_fullA1", tag="a1",
                                addr_space="Shared")]
            t_fullB = [dr4.tile([VTOT - LO_LIM, D], bf16, name="t_fullB0",
                                tag="b0", addr_space="Shared"),
                       dr4.tile([VTOT - LO_LIM, D], bf16, name="t_fullB1",
                                tag="b1", addr_space="Shared")]

            # zero all gather buffers once so count-trimmed (ungathered)
            # tail positions hold finite values (NaN*0 = NaN in the PE)
            for i in range(DEPTH + 1):
                gz = gp.tile([P, 2 * CLO * D], bf16, tag="glo", bufs=DEPTH + 1,
                             name=f"gz_lo{i}")
                nc.vector.memset(gz[:], 0)
            for i in range(HDEPTH + 2):
                gz = gp.tile([P, 2 * CHI * D], bf16, tag="ghi", bufs=HDEPTH + 2,
                             name=f"gz_hi{i}")
                nc.vector.memset(gz[:], 0)

            # ---- phase 0: XW1 ----
            xt_sb = xtp.tile([P, 4 * NLOC], bf16)
            for k in range(4):
                nc.sync.dma_start(xt_sb[:, k * NLOC:k * NLOC + ROWS_A],
                                  xt_d[k * P:(k + 1) * P, 0:ROWS_A])
            for k in range(4):
                nc.sync.dma_start(xt_sb[:, k * NLOC + ROWS_A:(k + 1) * NLOC],
                                  xt_d[k * P:(k + 1) * P, ROWS_A:NLOC])
            for t in range(TILES):
                ps = xwps.tile([P, D], f32, tag="xw")
                for k in range(4):
                    nc.tensor.matmul(
                        out=ps[:],
                        lhsT=xt_sb[:, k * NLOC + t * P: k * NLOC + (t + 1) * P],
                        rhs=w1_sb[:, k * D:(k + 1) * D],
                        start=(k == 0), stop=(k == 3),
                    )
                nc.scalar.activation(out=t_all[0][:, t * P:(t + 1) * P],
                                     in_=ps[:],
                                     func=mybir.ActivationFunctionType.Copy)
                nc.sync.dma_start(t_loc[0][t * P:(t + 1) * P, :],
                                  t_all[0][:, t * P:(t + 1) * P])

            def allgather(li):
                nc.gpsimd.collective_compute(
                    "AllGather",
                    mybir.AluOpType.bypass,
                    replica_groups=[list(range(NCORES))],
                    ins=[t_loc[li][0:ROWS_A, :].opt()],
                    outs=[t_fullA[li][:, :].opt()],
                )
                nc.gpsimd.collective_compute(
                    "AllGather",
                    mybir.AluOpType.bypass,
                    replica_groups=[list(range(NCORES))],
                    ins=[t_loc[li][ROWS_A:NLOC, :].opt()],
                    outs=[t_fullB[li][:, :].opt()],
                )

            def gather_half(li, pp, nt, half):
                tag = "glo" if half == 0 else "ghi"
                nbuf = DEPTH + 1 if half == 0 else HDEPTH + 2
                cap = LO_CAP if half == 0 else HI_CAP
                nch = CLO if half == 0 else CHI
                g = gp.tile([P, 2 * nch * D], bf16, tag=tag, bufs=nbuf,
                            name=f"g{tag}{li}_{pp}")
                ni = nt * cap
                src = (t_fullA[li][:, :] if half == 0
                       else t_fullB[li][:, :])
                c0 = pp * (LO_COLS + HI_COLS) + (0 if half == 0 else LO_COLS)
                qn = qctr[0] % 4
                ni_reg = cnt_regs[qn]
                nc.gpsimd.reg_load(ni_reg,
                                   cnt_sb[0:1, 2 * pp + half:2 * pp + half + 1])
                gi = nc.gpsimd.dma_gather(
                    g[:, :nt * nch * D].rearrange("p (c d) -> p c d", d=D),
                    src,
                    gidx_sb[:, c0:c0 + ni // 16],
                    ni, ni_reg, D, single_packet=False,
                    queue_num=qn,
                )
                qctr[0] += 1
                if first_gather[0]:
                    add_dep_helper(gi.ins, lib.ins, reason="lib before gather")
                    first_gather[0] = False
                return g

            def agg_tile(li, qq, ti, g_lo, g_hi):
                t = 2 * qq + ti
                s_sb = sp.tile([P, (CPT + 1) * P], bf16, tag="s", bufs=4)
                nc.sync.dma_start(s_sb[:], sval_d[t, :, :])
                agg = aggps.tile([P, P], f32, tag="agg")
                for c in range(CLO):
                    nc.tensor.matmul(
                        out=agg[:],
                        lhsT=g_lo[:, (ti * CLO + c) * D:(ti * CLO + c + 1) * D],
                        rhs=s_sb[:, c * P:(c + 1) * P],
                        start=(c == 0), stop=False,
                    )
                for c in range(CHI):
                    nc.tensor.matmul(
                        out=agg[:],
                        lhsT=g_hi[:, (ti * CHI + c) * D:(ti * CHI + c + 1) * D],
                        rhs=s_sb[:, (CLO + c) * P:(CLO + c + 1) * P],
                        start=False, stop=False,
                    )
                nc.tensor.matmul(
                    out=agg[:],
                    lhsT=t_all[li][:, t * P:(t + 1) * P],
                    rhs=s_sb[:, CPT * P:(CPT + 1) * P],
                    start=False, stop=True,
                )
                return agg

            # ---- phase 1+2: layer-1 aggregation + XW2 ----
            allgather(0)
            glo_buf = {}
            ghi_buf = {}
            for pp in range(PAIRS + DEPTH):
                nt_pp = 2 if pp < PAIRS and 2 * pp + 1 < TILES else 1
                if pp < PAIRS:
                    glo_buf[pp] = (gather_half(0, pp, nt_pp, 0), nt_pp)
                hh = pp - DEPTH + HDEPTH
                if 0 <= hh < PAIRS:
                    nt_hh = 2 if 2 * hh + 1 < TILES else 1
                    ghi_buf[hh] = gather_half(0, hh, nt_hh, 1)
                qq = pp - DEPTH
                if qq < 0:
                    continue
                g_lo, nt = glo_buf.pop(qq)
                g_hi = ghi_buf.pop(qq)
                for ti in range(nt):
                    t = 2 * qq + ti
                    agg = agg_tile(0, qq, ti, g_lo, g_hi)
                    h1t = hp.tile([P, P], bf16, tag="h")
                    nc.scalar.activation(out=h1t[:], in_=agg[:],
                                         func=mybir.ActivationFunctionType.Relu,
                                         bias=b1_sb[:])
                    ps2 = xwps.tile([P, D], f32, tag="xw2")
                    nc.tensor.matmul(out=ps2[:], lhsT=h1t[:], rhs=w2_sb[:],
                                     start=True, stop=True)
                    nc.vector.tensor_copy(out=t_all[1][:, t * P:(t + 1) * P],
                                          in_=ps2[:])
                    nc.sync.dma_start(t_loc[1][t * P:(t + 1) * P, :],
                                      t_all[1][:, t * P:(t + 1) * P])

            # ---- phase 3+4: layer-2 aggregation + head ----
            allgather(1)
            glo_buf = {}
            ghi_buf = {}
            for pp in range(PAIRS + DEPTH):
                nt_pp = 2 if pp < PAIRS and 2 * pp + 1 < TILES else 1
                if pp < PAIRS:
                    glo_buf[pp] = (gather_half(1, pp, nt_pp, 0), nt_pp)
                hh = pp - DEPTH + HDEPTH
                if 0 <= hh < PAIRS:
                    nt_hh = 2 if 2 * hh + 1 < TILES else 1
                    ghi_buf[hh] = gather_half(1, hh, nt_hh, 1)
                qq = pp - DEPTH
                if qq < 0:
                    continue
                g_lo, nt = glo_buf.pop(qq)
                g_hi = ghi_buf.pop(qq)
                for ti in range(nt):
                    t = 2 * qq + ti
                    agg = agg_tile(1, qq, ti, g_lo, g_hi)
                    h2t = hp.tile([P, P], bf16, tag="h")
                    nc.scalar.activation(out=h2t[:], in_=agg[:],
                                         func=mybir.ActivationFunctionType.Relu,
                                         bias=b2_sb[:])
                    lg = lgps.tile([P, NCLS], f32, tag="lg")
                    nc.tensor.matmul(out=lg[:], lhsT=h2t[:], rhs=wl_sb[:],
                                     start=True, stop=True)
                    l_sb = hdp.tile([P, NCLS], f32, tag="l")
                    nc.vector.tensor_add(out=l_sb[:], in0=lg[:], in1=bl_sb[:])
                    nmx = hdp.tile([P, 1], f32, tag="nmx")
                    nc.vector.reduce_max(out=nmx[:], in_=l_sb[:],
                                         axis=mybir.AxisListType.X, negate=True)
                    e_sb = hdp.tile([P, NCLS], f32, tag="e")
                    nc.scalar.activation(out=e_sb[:], in_=l_sb[:],
                                         func=mybir.ActivationFunctionType.Exp,
                                         bias=nmx[:])
                    sm = hdp.tile([P, 1], f32, tag="sm")
                    nc.vector.reduce_sum(out=sm[:], in_=e_sb[:],
                                         axis=mybir.AxisListType.X)
                    rs = hdp.tile([P, 1], f32, tag="rs")
                    nc.vector.reciprocal(out=rs[:], in_=sm[:])
                    pr = hdp.tile([P, NCLS], f32, tag="pr")
                    nc.scalar.activation(out=pr[:], in_=e_sb[:],
                                         func=mybir.ActivationFunctionType.Copy,
                                         scale=rs[:])
                    nc.sync.dma_start(out_d[t * P:(t + 1) * P, :], pr[:])

    nc.compile()
    return nc


def _preprocess(x, edge_index, W1, b1, W2, b2, Wlin, blin):
    """Host-side graph preprocessing -> per-core input dicts + slot maps."""
    x = np.asarray(x, np.float32)
    ei = np.asarray(edge_index)
    row = ei[0].astype(np.int64)
    col = ei[1].astype(np.int64)

    deg = np.bincount(col, minlength=N).astype(np.float32) + 2.0
    dis = 1.0 / np.sqrt(deg)
    norm_e = (dis[row] * dis[col]).astype(np.float32)
    selfval = (2.0 * dis * dis).astype(np.float32)

    indeg = np.bincount(col, minlength=N)  # per-node in-edges (no self)

    # balanced node->bin assignment (bins = core*TILES + tile), snake by degree
    NB = NCORES * TILES
    order = np.argsort(-indeg, kind="stable")
    bin_of_node = np.empty(N, np.int64)
    full_rounds = N // NB
    rem = N - full_rounds * NB
    fwd = np.arange(NB)
    bwd = fwd[::-1]
    seq = []
    for r in range(full_rounds):
        seq.append(fwd if r % 2 == 0 else bwd)
    if rem:
        seq.append((fwd if full_rounds % 2 == 0 else bwd)[:rem])
    seq = np.concatenate(seq)
    bin_of_node[order] = seq

    # within each pair of tiles, put the tile with more in-edges first
    bin_edges = np.bincount(bin_of_node[col], minlength=NB)
    perm = np.arange(NB)
    for c in range(NCORES):
        for pp in range(TILES // 2):
            b0 = c * TILES + 2 * pp
            if bin_edges[b0] < bin_edges[b0 + 1]:
                perm[b0], perm[b0 + 1] = b0 + 1, b0
    inv = np.empty(NB, np.int64)
    inv[perm] = np.arange(NB)
    bin_of_node = inv[bin_of_node]

    pos_in_bin = np.empty(N, np.int64)
    srt = np.argsort(bin_of_node, kind="stable")
    cnt = np.bincount(bin_of_node, minlength=NB)
    assert cnt.max() <= P
    starts = np.zeros(NB + 1, np.int64)
    np.cumsum(cnt, out=starts[1:])
    pos_in_bin[srt] = np.arange(N) - starts[bin_of_node[srt]]

    bin_edge_cnt = np.bincount(bin_of_node[col], minlength=NB)
    assert bin_edge_cnt.max() <= ECAP, (
        f"bin edge overflow: {bin_edge_cnt.max()} > {ECAP}")

    core_of_node = bin_of_node // TILES
    tile_of_node = bin_of_node % TILES
    # table row: AG-A region holds tiles 0..TILES_A-1 of every core, then AG-B
    gslot = np.where(
        tile_of_node < TILES_A,
        core_of_node * ROWS_A + tile_of_node * P + pos_in_bin,
        LO_LIM + core_of_node * ROWS_B + (tile_of_node - TILES_A) * P + pos_in_bin,
    )

    # per-edge: destination bin + dest position; source table slot
    e_bin = bin_of_node[col]
    e_dpos = pos_in_bin[col]
    e_src = gslot[row]

    # group edges by bin
    e_order = np.argsort(e_bin, kind="stable")
    eb = e_bin[e_order]
    ed = e_dpos[e_order]
    es = e_src[e_order]
    en = norm_e[e_order]
    bstarts = np.searchsorted(eb, np.arange(NB + 1))

    in_maps = []
    for c in range(NCORES):
        gidx = np.zeros((P, PAIRS * (LO_COLS + HI_COLS)), np.int16)
        sval_f32 = np.zeros((TILES, P, (CPT + 1) * P), np.float32)
        cntv = np.ones((2 * PAIRS,), np.int32)
        for pp in range(PAIRS):
            nt = 2 if 2 * pp + 1 < TILES else 1
            # per tile: (rel_idx, dpos, norm) for lo and hi halves
            halves = {0: [], 1: []}
            for ti in range(nt):
                t = 2 * pp + ti
                b = c * TILES + t
                lo_f, hi_f = bstarts[b], bstarts[b + 1]
                srcs = es[lo_f:hi_f]
                dposs = ed[lo_f:hi_f]
                nrm = en[lo_f:hi_f]
                ne = len(srcs)
                is_lo = srcs < LO_LIM
                lo_n = int(is_lo.sum())
                assert lo_n <= LO_CAP and ne - lo_n <= HI_CAP, (c, t, ne, lo_n)
                for half in (0, 1):
                    sel = is_lo if half == 0 else ~is_lo
                    hs, hd, hn = srcs[sel], dposs[sel], nrm[sel]
                    rel = hs if half == 0 else hs - LO_LIM
                    o3 = np.argsort(rel, kind="stable")
                    rel, hd, hn = rel[o3], hd[o3], hn[o3]
                    # dedup repeated sources: gather once, S row gets all the
                    # (dest, norm) entries of the duplicates
                    uniq, uinv = np.unique(rel, return_inverse=True)
                    halves[half].append((uniq, uinv, hd, hn))
            for half in (0, 1):
                cap = LO_CAP if half == 0 else HI_CAP
                flat = np.zeros(nt * cap, np.int64)
                last_k = 0
                for ti in range(nt):
                    t = 2 * pp + ti
                    uniq, uinv, hd, hn = halves[half][ti]
                    k = len(uniq)
                    last_k = k
                    flat[ti * cap: ti * cap + k] = uniq
                    if k < cap:
                        # interior padding: repeat last valid idx (or 0)
                        flat[ti * cap + k: (ti + 1) * cap] = uniq[-1] if k else 0
                    # dense S values: tile-local position -> (chunk, partition)
                    cbase = 0 if half == 0 else CLO
                    cidx = cbase + uinv // P
                    pidx = uinv % P
                    np.add.at(sval_f32, (np.full(len(hd), t), pidx,
                                         cidx * P + hd), hn)
                cnt_ph = max(1, (nt - 1) * cap + last_k)
                cntv[2 * pp + half] = cnt_ph
                # trailing padding past the real count is -1: the ucode trims
                # it and the decode reserves ring space from the count reg,
                # so both sides agree on the descriptor count
                flat[cnt_ph:] = -1
                w = flat.reshape(len(flat) // 16, 16).T.astype(np.int16)
                c0 = pp * (LO_COLS + HI_COLS) + (0 if half == 0 else LO_COLS)
                gidx[:, c0:c0 + len(flat) // 16] = np.tile(w, (8, 1))
        # self-loop diagonal chunk (chunk CPT of each tile)
        mine = np.where(core_of_node == c)[0]
        lslot = tile_of_node[mine] * P + pos_in_bin[mine]
        for t in range(TILES):
            sel = tile_of_node[mine] == t
            nodes_t = mine[sel]
            pos_t = pos_in_bin[nodes_t]
            sval_f32[t, pos_t, CPT * P + pos_t] = selfval[nodes_t]
        sval = sval_f32.astype(ml_dtypes.bfloat16)
        # x slice, transposed, padded
        xt = np.zeros((FIN, NLOC), ml_dtypes.bfloat16)
        xt[:, lslot] = x[mine].T.astype(ml_dtypes.bfloat16)
        in_maps.append({
            "xt": xt,
            "w1": np.asarray(W1).astype(ml_dtypes.bfloat16),
            "w2": np.asarray(W2).astype(ml_dtypes.bfloat16),
            "wl": np.asarray(Wlin).astype(ml_dtypes.bfloat16),
            "b1": np.asarray(b1, np.float32).reshape(P, 1),
            "b2": np.asarray(b2, np.float32).reshape(P, 1),
            "bl": np.tile(np.asarray(blin, np.float32).reshape(1, NCLS), (P, 1)),
            "gidx": gidx,
            "sval": sval,
            "cnt": np.tile(cntv[None, :], (P, 1)),
        })
    return in_maps, core_of_node, tile_of_node, pos_in_bin


def kernel(x, edge_index, W1, b1, W2, b2, Wlin, blin):
    global _PROGRAM, LAST_EXEC_NS, LAST_RESULT
    in_maps, core_of, tile_of, pos_of = _preprocess(
        x, edge_index, W1, b1, W2, b2, Wlin, blin)
    if _PROGRAM is None:
        _PROGRAM = _build_program()
    res = run_bass_kernel_spmd(
        _PROGRAM, in_maps, core_ids=list(range(NCORES)), trace=TRACE)
    LAST_EXEC_NS = res.exec_time_ns
    LAST_RESULT = res
    out = np.empty((N, NCLS), np.float32)
    per_core = [res.results[c]["probs"] for c in range(NCORES)]
    lslot = tile_of * P + pos_of
    for c in range(NCORES):
        mine = np.where(core_of == c)[0]
        out[mine] = per_core[c][lslot[mine]]
    return out
